# revision 3
# baseline (speedup 1.0000x reference)
"""GIN encoder (3x GINConv+BN + per-layer global_add_pool) on 8 TRN2 cores.

v3: sharded design. Each core owns one segment of N/8 nodes (104 tiles of
125 seats) and the edges incident to them; per layer it gathers neighbor
rows from a full replicated node table in its DRAM, computes the GIN MLP,
and contributes its shard of the next layer's table via one AllGather
(3.4MB -> 27MB bf16, ~100us). BatchNorm statistics are all-reduced
([128,2] per layer) and folded lazily into the next layer's MLP exactly as
in v2: with h = a*z + c (per-feature), the GIN input
h_i + sum_j h_j = a*(z_i + sum z_j) + c*(1+deg_i), so
  mp1 = (diag(a) W1)^T t + (W1^T c) (x) (1+deg)   [rank-1 via K=1 matmul]
and pools are fixed post-hoc: pool_bn = a*pool_raw + c (x) count_g.
Per-core pools (own segment only) are combined on the host.

The feature-major self-term table and the gather indices live entirely in
SBUF; only the node-major bf16 gather table (written by AllGather) is in
DRAM.
"""
import sys
sys.path.insert(0, '/opt/trn_rl_repo')

import numpy as np
import ml_dtypes

import concourse.bass as bass
import concourse.tile as tile
from concourse import bacc, mybir, library_config
from concourse import bass_utils

NCORES = 8
N = 100000
F = 128
E = 1600000
L = 3
NUM_GRAPHS = 512
BN_EPS = 1e-5
P = 128

SEG = 8                 # segments == cores
NPS = N // SEG          # nodes per segment (12500)
T = 104                 # dst tiles per segment
SEATS = 125             # real seats per tile
CAPB = 512              # slots per (tile, bank)
BCPT = CAPB // 128      # 128-chunks per (tile, bank)
G = 1                   # tiles per gather batch
GRP = T // G            # gather groups per core
IDXC = (G * CAPB) // 16  # gidx cols per call
COLS = T * P            # padded node columns per segment (13312)
ROWS = SEG * COLS       # full table rows (106496)
NBANK = 4
BANK_ROWS = ROWS // NBANK
assert BANK_ROWS < 32767 and T % G == 0 and T * SEATS >= NPS


def preprocess(x, edge_index, batch):
    src = edge_index[0].astype(np.int64)
    dst = edge_index[1].astype(np.int64)
    seg_of = dst // NPS
    src_bank = src // (2 * NPS)

    tile_of_g = np.empty(N, np.int64)
    seat_of_g = np.empty(N, np.int64)
    for s in range(SEG):
        m = seg_of == s
        d_loc = dst[m] - s * NPS
        counts = np.zeros((NPS, 4), np.int64)
        np.add.at(counts, (d_loc, src_bank[m]), 1)
        deg = counts.sum(1)
        order = np.argsort(-deg, kind='stable')
        rem = np.full((T, 4), CAPB, np.int64)
        seats = np.full(T, SEATS, np.int64)
        tile_of = np.full(NPS, -1, np.int64)
        seat_of = np.full(NPS, -1, np.int64)
        for d in order:
            v = counts[d]
            feas = (seats > 0) & (rem >= v).all(1)
            assert feas.any(), "tile packing failed"
            slack = (rem - v).min(1).astype(np.float64)
            slack[~feas] = -1e18
            t = int(np.argmax(slack))
            tile_of[d] = t
            seat_of[d] = SEATS - seats[t]
            rem[t] -= v
            seats[t] -= 1
        tile_of_g[s * NPS:(s + 1) * NPS] = tile_of
        seat_of_g[s * NPS:(s + 1) * NPS] = seat_of

    perm_pos = tile_of_g * P + seat_of_g                  # pos within segment
    perm_row = (np.arange(N) // NPS) * COLS + perm_pos    # global table row

    xf = np.asarray(x, np.float32)
    x_nm = np.zeros((ROWS, F), np.float32)
    x_nm[perm_row] = xf

    indeg = np.bincount(dst, minlength=N).astype(np.float32)
    d_all = np.zeros(ROWS, np.float32)
    d_all[perm_row] = 1.0 + indeg

    in_maps, metas = [], []
    for s in range(SEG):
        lo, hi = s * NPS, (s + 1) * NPS
        g0 = int(batch[lo]); sp = int(batch[hi - 1]) - g0 + 1
        assert sp <= P
        rl, rh = s * COLS, (s + 1) * COLS

        brel = np.full(COLS, -1.0, np.float32)
        brel[perm_pos[lo:hi]] = (batch[lo:hi] - g0).astype(np.float32)
        brel_h = np.ascontiguousarray(brel.reshape(T, P).T)          # [128, T]

        filled_row = np.bincount(tile_of_g[lo:hi], minlength=T).astype(np.float32)
        filled_h = np.tile(filled_row, (P, 1)).astype(np.float32)    # [128, T]

        gcnt = np.bincount(batch[lo:hi] - g0, minlength=P).astype(np.float32)
        cnt_h = gcnt.reshape(1, P)

        # edge slots for this core: key = (tile, bank)
        m = seg_of == s
        key = tile_of_g[dst[m]] * 4 + src_bank[m]
        order_e = np.argsort(key, kind='stable')
        key_s = key[order_e]
        cnt_e = np.bincount(key_s, minlength=T * 4)
        assert cnt_e.max() <= CAPB
        cstart = np.zeros(T * 4, np.int64)
        cstart[1:] = np.cumsum(cnt_e)[:-1]
        ne = int(m.sum())
        within = np.arange(ne) - np.repeat(cstart, cnt_e)
        slot = key_s * CAPB + within
        gidx_flat = np.zeros(T * 4 * CAPB, np.int64)
        drel_flat = np.full(T * 4 * CAPB, -1.0, np.float32)
        src_s = src[m][order_e]
        dst_s = dst[m][order_e]
        gidx_flat[slot] = perm_row[src_s] % BANK_ROWS
        drel_flat[slot] = seat_of_g[dst_s]

        # gidx repacked for batched calls: call = (grp, bank) over G tiles
        gi = gidx_flat.reshape(GRP, G, 4, CAPB)
        gi = gi.transpose(0, 2, 1, 3).reshape(GRP * 4, G * CAPB)
        w = gi.reshape(-1, (G * CAPB) // 16, 16).transpose(0, 2, 1)
        w = w.reshape(GRP * 4, 16, (G * CAPB) // 16)
        w = np.concatenate([w[i] for i in range(w.shape[0])], axis=1)
        gidx_h = np.tile(w, (8, 1)).astype(np.int16)     # [128, GRP*4*IDXC]
        drel_h = np.ascontiguousarray(
            drel_flat.reshape(T * 4 * BCPT, P).T)        # [128, T*4*BCPT]

        in_maps.append(dict(
            xsh=x_nm[rl:rh].astype(ml_dtypes.bfloat16),
            xfm=np.ascontiguousarray(
                x_nm[rl:rh].T).astype(ml_dtypes.bfloat16),
            gidx=gidx_h, dstrel=drel_h, brel=brel_h, filled=filled_h,
            drow=d_all[rl:rh].reshape(1, COLS),
            cnt=cnt_h,
        ))
        metas.append(dict(g_base=g0, span=sp))
    return in_maps, metas


def build_kernel(repeat=1):
    dt = mybir.dt
    nc = bacc.Bacc("TRN2", target_bir_lowering=False, debug=False,
                   enable_asserts=False, num_devices=NCORES,
                   num_swdge_queues=4)

    xsh_d = nc.dram_tensor("xsh", [COLS, F], dt.bfloat16, kind="ExternalInput")
    xfm_d = nc.dram_tensor("xfm", [P, COLS], dt.bfloat16, kind="ExternalInput")
    gidx_d = nc.dram_tensor("gidx", [P, GRP * 4 * IDXC], dt.int16,
                            kind="ExternalInput")
    drel_d = nc.dram_tensor("dstrel", [P, T * 4 * BCPT], dt.float32,
                            kind="ExternalInput")
    brel_d = nc.dram_tensor("brel", [P, T], dt.float32, kind="ExternalInput")
    fill_d = nc.dram_tensor("filled", [P, T], dt.float32, kind="ExternalInput")
    drow_d = nc.dram_tensor("drow", [1, COLS], dt.float32, kind="ExternalInput")
    cnt_d = nc.dram_tensor("cnt", [1, P], dt.float32, kind="ExternalInput")
    w1_d = nc.dram_tensor("w1", [P, L * F], dt.float32, kind="ExternalInput")
    w2_d = nc.dram_tensor("w2", [P, L * F], dt.float32, kind="ExternalInput")
    bias_d = nc.dram_tensor("bias", [P, 4 * L], dt.float32, kind="ExternalInput")
    iota_d = nc.dram_tensor("iotat", [P, P], dt.float32, kind="ExternalInput")
    ident_d = nc.dram_tensor("ident", [P, P], dt.float32, kind="ExternalInput")
    pools_d = nc.dram_tensor("pools", [P, L * F], dt.float32,
                             kind="ExternalOutput")

    # full node tables (AllGather outputs; ping-pong across layers)
    tabs = [nc.dram_tensor(f"tab{i}", [ROWS, F], dt.bfloat16, kind="Internal",
                           addr_space="Shared") for i in range(2)]
    # own-shard bounce buffers (AllGather inputs)
    sh_x = nc.dram_tensor("shx", [COLS, F], dt.bfloat16, kind="Internal")
    shs = [nc.dram_tensor(f"sh{l}", [COLS, F], dt.bfloat16, kind="Internal")
           for l in range(L - 1)]
    stat_in = [nc.dram_tensor(f"sin{l}", [P, 2], dt.float32, kind="Internal")
               for l in range(L)]
    stat_out = [nc.dram_tensor(f"sout{l}", [P, 2], dt.float32, kind="Internal",
                               addr_space="Shared") for l in range(L)]

    inv_n = 1.0 / N
    grp8 = [list(range(NCORES))]

    with tile.TileContext(nc) as tc:
        with tc.tile_pool(name="big", bufs=1) as big, \
             tc.tile_pool(name="gpool", bufs=2) as gpool, \
             tc.tile_pool(name="spool", bufs=8) as spool, \
             tc.tile_pool(name="work", bufs=4) as work, \
             tc.tile_pool(name="stat", bufs=1) as statp, \
             tc.tile_pool(name="psA", bufs=2, space="PSUM") as psA, \
             tc.tile_pool(name="psM", bufs=2, space="PSUM") as psM, \
             tc.tile_pool(name="psT", bufs=2, space="PSUM") as psT, \
             tc.tile_pool(name="psP", bufs=1, space="PSUM") as psP:

            nc.gpsimd.load_library(library_config.mlp)

            gidx_t = big.tile([P, GRP * 4 * IDXC], dt.int16)
            nc.sync.dma_start(gidx_t[:], gidx_d.ap())
            drel_t = big.tile([P, T * 4 * BCPT], dt.float32)
            nc.sync.dma_start(drel_t[:], drel_d.ap())
            brel_t = big.tile([P, T], dt.float32)
            nc.sync.dma_start(brel_t[:], brel_d.ap())
            fill_t = big.tile([P, T], dt.float32)
            nc.sync.dma_start(fill_t[:], fill_d.ap())
            drow_t = big.tile([1, COLS], dt.float32)
            nc.sync.dma_start(drow_t[:], drow_d.ap())
            cnt_t = big.tile([1, P], dt.float32)
            nc.sync.dma_start(cnt_t[:], cnt_d.ap())
            w1_t = big.tile([P, L * F], dt.float32)
            nc.sync.dma_start(w1_t[:], w1_d.ap())
            w2_t = big.tile([P, L * F], dt.float32)
            nc.sync.dma_start(w2_t[:], w2_d.ap())
            bias_t = big.tile([P, 4 * L], dt.float32)
            nc.sync.dma_start(bias_t[:], bias_d.ap())
            iota_t = big.tile([P, P], dt.float32)
            nc.sync.dma_start(iota_t[:], iota_d.ap())
            ident_t = big.tile([P, P], dt.float32)
            nc.sync.dma_start(ident_t[:], ident_d.ap())

            # SBUF-resident feature-major self-term tables (ping-pong)
            hfm = [big.tile([P, COLS], dt.bfloat16, name=f"hfm{i}")
                   for i in range(2)]
            nc.sync.dma_start(hfm[0][:], xfm_d.ap())
            # stage own x shard for the layer-0 AllGather
            nc.sync.dma_start(sh_x.ap(), xsh_d.ap())

            # folded W1 and rank-1 rows for layers 1,2 (layer 0: a=1, c=0)
            w1f_t = big.tile([P, (L - 1) * F], dt.float32)
            w1c_t = big.tile([1, (L - 1) * F], dt.float32)
            ac_t = big.tile([P, 2 * L], dt.float32)
            cr_t = big.tile([1, L * F], dt.float32)

            for rep in range(repeat):
              nc.gpsimd.collective_compute(
                  "AllGather", mybir.AluOpType.bypass, replica_groups=grp8,
                  ins=[sh_x.ap().opt()], outs=[tabs[0].ap().opt()])
              for l in range(L):
                tab = tabs[l % 2]
                hcur = hfm[l % 2]
                hnxt = hfm[(l + 1) % 2]
                b1c = bias_t[:, 0 * L + l:0 * L + l + 1]
                b2c = bias_t[:, 1 * L + l:1 * L + l + 1]
                gac = bias_t[:, 2 * L + l:2 * L + l + 1]
                bec = bias_t[:, 3 * L + l:3 * L + l + 1]
                w2c = w2_t[:, l * F:(l + 1) * F]
                w1c = (w1_t[:, 0:F] if l == 0
                       else w1f_t[:, (l - 1) * F:l * F])

                ssum = statp.tile([P, T], dt.float32, tag=f"ssum{l}")
                ssq = statp.tile([P, T], dt.float32, tag=f"ssq{l}")
                pool_ps = psP.tile([P, P], dt.float32, tag="pool")

                for g in range(GRP):
                    g_t = gpool.tile([P, 4, G * BCPT, P], dt.bfloat16, tag="G")
                    call0 = g * 4
                    for b in range(4):
                        nc.gpsimd.dma_gather(
                            out_ap=g_t[:, b],
                            in_ap=tab.ap()[b * BANK_ROWS:(b + 1) * BANK_ROWS, :],
                            idxs_ap=gidx_t[:, (call0 + b) * IDXC:
                                           (call0 + b + 1) * IDXC],
                            num_idxs=G * CAPB,
                            num_idxs_reg=G * CAPB,
                            elem_size=F,
                            queue_num=b,
                        )
                    for ti in range(G):
                        t = g * G + ti
                        aggT = psA.tile([P, P], dt.float32, tag="agg")
                        for b in range(4):
                            for sub in range(BCPT):
                                ch = t * 16 + b * BCPT + sub
                                k = ti * BCPT + sub
                                s_t = spool.tile([P, P], dt.bfloat16, tag="S")
                                nc.vector.tensor_scalar(
                                    out=s_t[:], in0=iota_t[:],
                                    scalar1=drel_t[:, ch:ch + 1],
                                    scalar2=None,
                                    op0=mybir.AluOpType.is_equal)
                                nc.tensor.matmul(
                                    aggT[:], lhsT=g_t[:, b, k, :],
                                    rhs=s_t[:],
                                    start=(b == 0 and sub == 0),
                                    stop=(b == 3 and sub == BCPT - 1))
                        z1in = work.tile([P, P], dt.float32, tag="z1in")
                        nc.vector.tensor_add(
                            z1in[:], hcur[:, t * P:(t + 1) * P], aggT[:])
                        mp1 = psM.tile([P, P], dt.float32, tag="mp")
                        nc.tensor.matmul(mp1[:], lhsT=w1c, rhs=z1in[:],
                                         start=True, stop=(l == 0))
                        if l > 0:
                            nc.tensor.matmul(
                                mp1[:],
                                lhsT=w1c_t[:, (l - 1) * F:l * F],
                                rhs=drow_t[0:1, t * P:(t + 1) * P],
                                start=False, stop=True)
                        z1 = work.tile([P, P], dt.float32, tag="z1")
                        nc.scalar.activation(
                            z1[:], mp1[:],
                            mybir.ActivationFunctionType.Relu, bias=b1c)
                        mp2 = psM.tile([P, P], dt.float32, tag="mp")
                        nc.tensor.matmul(mp2[:], lhsT=w2c, rhs=z1[:],
                                         start=True, stop=True)
                        zf = work.tile([P, P], dt.float32, tag="zf")
                        nc.scalar.activation(
                            zf[:], mp2[:],
                            mybir.ActivationFunctionType.Relu, bias=b2c)
                        # zero phantom seats, then stats
                        msk = spool.tile([P, P], dt.float32, tag="M")
                        nc.vector.tensor_scalar(
                            out=msk[:], in0=iota_t[:],
                            scalar1=fill_t[:, t:t + 1], scalar2=None,
                            op0=mybir.AluOpType.is_lt)
                        nc.vector.tensor_tensor(
                            out=zf[:], in0=zf[:], in1=msk[:],
                            op=mybir.AluOpType.mult)
                        nc.vector.tensor_reduce(
                            out=ssum[:, t:t + 1], in_=zf[:],
                            axis=mybir.AxisListType.X,
                            op=mybir.AluOpType.add)
                        sqs = work.tile([P, P], dt.float32, tag="sqs")
                        nc.scalar.activation(
                            sqs[:], zf[:],
                            mybir.ActivationFunctionType.Square,
                            accum_out=ssq[:, t:t + 1])
                        if l < L - 1:
                            nc.vector.tensor_copy(
                                hnxt[:, t * P:(t + 1) * P], zf[:])
                        zT = psT.tile([P, P], dt.float32, tag="tt")
                        nc.tensor.transpose(zT[:], zf[:], ident_t[:])
                        znm = work.tile([P, P], dt.bfloat16, tag="znm")
                        nc.vector.tensor_copy(znm[:], zT[:])
                        if l < L - 1:
                            nc.sync.dma_start(
                                shs[l].ap()[t * P:(t + 1) * P, :], znm[:])
                        sb_t = spool.tile([P, P], dt.bfloat16, tag="S")
                        nc.vector.tensor_scalar(
                            out=sb_t[:], in0=iota_t[:],
                            scalar1=brel_t[:, t:t + 1], scalar2=None,
                            op0=mybir.AluOpType.is_equal)
                        nc.tensor.matmul(pool_ps[:], lhsT=sb_t[:],
                                         rhs=znm[:],
                                         start=(t == 0), stop=(t == T - 1),
                                         skip_group_check=True)

                # raw pool (graph-major) for this layer
                prm = statp.tile([P, P], dt.float32, tag=f"prm{l}")
                nc.scalar.copy(prm[:], pool_ps[:])

                # ---- global BN stats via AllReduce -> a, c; fold ----
                red = work.tile([P, 2], dt.float32, tag="red")
                nc.vector.tensor_reduce(out=red[:, 0:1], in_=ssum[:],
                                        axis=mybir.AxisListType.X,
                                        op=mybir.AluOpType.add)
                nc.vector.tensor_reduce(out=red[:, 1:2], in_=ssq[:],
                                        axis=mybir.AxisListType.X,
                                        op=mybir.AluOpType.add)
                nc.sync.dma_start(stat_in[l].ap(), red[:])
                nc.gpsimd.collective_compute(
                    "AllReduce", mybir.AluOpType.add, replica_groups=grp8,
                    ins=[stat_in[l].ap().opt()],
                    outs=[stat_out[l].ap().opt()])
                # kick off the next layer's table exchange right behind it
                if l < L - 1:
                    nc.gpsimd.collective_compute(
                        "AllGather", mybir.AluOpType.bypass,
                        replica_groups=grp8,
                        ins=[shs[l].ap().opt()],
                        outs=[tabs[(l + 1) % 2].ap().opt()])
                gred = work.tile([P, 2], dt.float32, tag="gred")
                nc.sync.dma_start(gred[:], stat_out[l].ap())

                mean = work.tile([P, 1], dt.float32, tag="mean")
                nc.vector.tensor_scalar(out=mean[:], in0=gred[:, 0:1],
                                        scalar1=inv_n, scalar2=None,
                                        op0=mybir.AluOpType.mult)
                var = work.tile([P, 1], dt.float32, tag="var")
                nc.vector.tensor_scalar(out=var[:], in0=gred[:, 1:2],
                                        scalar1=inv_n, scalar2=None,
                                        op0=mybir.AluOpType.mult)
                msq = work.tile([P, 1], dt.float32, tag="msq")
                nc.vector.tensor_tensor(out=msq[:], in0=mean[:], in1=mean[:],
                                        op=mybir.AluOpType.mult)
                nc.vector.tensor_tensor(out=var[:], in0=var[:], in1=msq[:],
                                        op=mybir.AluOpType.subtract)
                nc.vector.tensor_scalar(out=var[:], in0=var[:],
                                        scalar1=BN_EPS, scalar2=None,
                                        op0=mybir.AluOpType.add)
                sd = work.tile([P, 1], dt.float32, tag="sd")
                nc.scalar.activation(sd[:], var[:],
                                     mybir.ActivationFunctionType.Sqrt)
                inv = work.tile([P, 1], dt.float32, tag="inv")
                nc.vector.reciprocal(inv[:], sd[:])
                a_c = ac_t[:, 2 * l:2 * l + 1]
                c_c = ac_t[:, 2 * l + 1:2 * l + 2]
                nc.vector.tensor_tensor(out=a_c, in0=inv[:], in1=gac,
                                        op=mybir.AluOpType.mult)
                tmpc = work.tile([P, 1], dt.float32, tag="tmpc")
                nc.vector.tensor_tensor(out=tmpc[:], in0=mean[:], in1=a_c,
                                        op=mybir.AluOpType.mult)
                nc.vector.tensor_tensor(out=c_c, in0=bec, in1=tmpc[:],
                                        op=mybir.AluOpType.subtract)
                # c as a single-partition row (for rank-1 matmuls)
                crow_ps = psT.tile([P, P], dt.float32, tag="tt")
                nc.tensor.matmul(crow_ps[0:1, :], lhsT=c_c, rhs=ident_t[:],
                                 start=True, stop=True)
                nc.scalar.copy(cr_t[:, l * F:(l + 1) * F], crow_ps[0:1, :])
                if l < L - 1:
                    # W1' = diag(a) W1_{l+1};  w1c_row = (W1_{l+1}^T c)^T
                    nc.scalar.activation(
                        w1f_t[:, l * F:(l + 1) * F],
                        w1_t[:, (l + 1) * F:(l + 2) * F],
                        mybir.ActivationFunctionType.Identity,
                        scale=a_c)
                    w1cc = psT.tile([P, P], dt.float32, tag="tt")
                    nc.tensor.matmul(w1cc[:, 0:1],
                                     lhsT=w1_t[:, (l + 1) * F:(l + 2) * F],
                                     rhs=c_c, start=True, stop=True)
                    w1cs = work.tile([P, 1], dt.float32, tag="w1cs")
                    nc.scalar.copy(w1cs[:], w1cc[:, 0:1])
                    w1cr = psT.tile([P, P], dt.float32, tag="tt")
                    nc.tensor.matmul(w1cr[0:1, :], lhsT=w1cs[:],
                                     rhs=ident_t[:], start=True, stop=True)
                    nc.scalar.copy(w1c_t[:, l * F:(l + 1) * F], w1cr[0:1, :])

                # ---- fix pool: a*poolT + c (x) cnt, back to [graph, feat] ----
                pT = psT.tile([P, P], dt.float32, tag="tt")
                nc.tensor.transpose(pT[:], prm[:], ident_t[:])
                u = work.tile([P, P], dt.float32, tag="u")
                nc.scalar.activation(u[:], pT[:],
                                     mybir.ActivationFunctionType.Identity,
                                     scale=ac_t[:, 2 * l:2 * l + 1])
                cc_ps = psM.tile([P, P], dt.float32, tag="mp")
                nc.tensor.matmul(cc_ps[:],
                                 lhsT=cr_t[:, l * F:(l + 1) * F],
                                 rhs=cnt_t[0:1, :],
                                 start=True, stop=True)
                nc.vector.tensor_add(u[:], u[:], cc_ps[:])
                pb_ps = psT.tile([P, P], dt.float32, tag="tt")
                nc.tensor.transpose(pb_ps[:], u[:], ident_t[:])
                pool_sb = work.tile([P, P], dt.float32, tag="poolsb")
                nc.scalar.copy(pool_sb[:], pb_ps[:])
                nc.sync.dma_start(
                    pools_d.ap()[:, l * F:(l + 1) * F], pool_sb[:])

    nc.compile()
    return nc


def make_in_maps(ins, inputs):
    W1 = np.asarray(inputs['W1'], np.float32)
    W2 = np.asarray(inputs['W2'], np.float32)
    b1 = np.asarray(inputs['b1'], np.float32)
    b2 = np.asarray(inputs['b2'], np.float32)
    gamma = np.asarray(inputs['gamma'], np.float32)
    beta = np.asarray(inputs['beta'], np.float32)
    w1_h = np.ascontiguousarray(np.concatenate([W1[i] for i in range(L)], 1))
    w2_h = np.ascontiguousarray(np.concatenate([W2[i] for i in range(L)], 1))
    bias_h = np.ascontiguousarray(
        np.concatenate([b1.T, b2.T, gamma.T, beta.T], 1))
    iota_h = np.tile(np.arange(P, dtype=np.float32), (P, 1))
    ident_h = np.eye(P, dtype=np.float32)
    shared = {"w1": w1_h, "w2": w2_h, "bias": bias_h,
              "iotat": iota_h, "ident": ident_h}
    return [{**ins[c], **shared} for c in range(NCORES)]


def kernel(x, edge_index, batch, W1, b1, W2, b2, gamma, beta):
    x = np.asarray(x, np.float32)
    edge_index = np.asarray(edge_index, np.int32)
    batch = np.asarray(batch, np.int32)

    ins, metas = preprocess(x, edge_index, batch)
    nc = build_kernel()
    in_maps = make_in_maps(ins, dict(W1=W1, W2=W2, b1=b1, b2=b2,
                                     gamma=gamma, beta=beta))

    import time as _time
    last_exc = None
    for attempt in range(3):
        try:
            res = bass_utils.run_bass_kernel_spmd(
                nc, in_maps, core_ids=list(range(NCORES)))
            break
        except Exception as e:
            last_exc = e
            _time.sleep(20)
    else:
        raise last_exc

    out = np.zeros((NUM_GRAPHS, L * F), np.float32)
    for c in range(NCORES):
        pools = res.results[c]["pools"]              # [128, L*F]
        g0, sp = metas[c]['g_base'], metas[c]['span']
        out[g0:g0 + sp] += pools[:sp]
    return out


if __name__ == "__main__":
    import reference
    inputs = reference.setup_inputs()
    inputs = {k: np.asarray(v) for k, v in inputs.items()}
    got = kernel(**inputs)
    print("kernel output shape:", got.shape)


# revision 51
# speedup vs baseline: 1.1462x; 1.1462x over previous
"""GIN encoder (3x GINConv+BN + per-layer global_add_pool) on 8 TRN2 cores.

v3: sharded design. Each core owns one segment of N/8 nodes (104 tiles of
125 seats) and the edges incident to them; per layer it gathers neighbor
rows from a full replicated node table in its DRAM, computes the GIN MLP,
and contributes its shard of the next layer's table via one AllGather
(3.4MB -> 27MB bf16, ~100us). BatchNorm statistics are all-reduced
([128,2] per layer) and folded lazily into the next layer's MLP exactly as
in v2: with h = a*z + c (per-feature), the GIN input
h_i + sum_j h_j = a*(z_i + sum z_j) + c*(1+deg_i), so
  mp1 = (diag(a) W1)^T t + (W1^T c) (x) (1+deg)   [rank-1 via K=1 matmul]
and pools are fixed post-hoc: pool_bn = a*pool_raw + c (x) count_g.
Per-core pools (own segment only) are combined on the host.

The feature-major self-term table and the gather indices live entirely in
SBUF; only the node-major bf16 gather table (written by AllGather) is in
DRAM.
"""
import sys
sys.path.insert(0, '/opt/trn_rl_repo')

import numpy as np
import ml_dtypes

import concourse.bass as bass
import concourse.tile as tile
from concourse import bacc, mybir, library_config
from concourse import bass_utils

NCORES = 8
N = 100000
F = 128
E = 1600000
L = 3
NUM_GRAPHS = 512
BN_EPS = 1e-5
P = 128

SEG = 8                 # segments == cores
NPS = N // SEG          # nodes per segment (12500)
T = 104                 # dst tiles per segment
SEATS = 125             # real seats per tile
CAPB = 512              # slots per (tile, bank)
BCPT = CAPB // 128      # 128-chunks per (tile, bank)
G = 2                   # tiles per gather batch
GRP = T // G            # gather groups per core
IDXC = (G * CAPB) // 16  # gidx cols per call
COLS = T * P            # padded node columns per segment (13312)
SHR = COLS + 4          # shard rows: nodes + 4 bf16 stat rows (hi/lo f32 split)
ROWS = SEG * SHR        # full table rows
NBANK = 4
BANK_ROWS = ROWS // NBANK
assert BANK_ROWS < 32767 and T % G == 0 and T * SEATS >= NPS


def preprocess(x, edge_index, batch):
    src = edge_index[0].astype(np.int64)
    dst = edge_index[1].astype(np.int64)
    seg_of = dst // NPS
    src_bank = src // (2 * NPS)

    tile_of_g = np.empty(N, np.int64)
    seat_of_g = np.empty(N, np.int64)
    for s in range(SEG):
        m = seg_of == s
        d_loc = dst[m] - s * NPS
        counts = np.zeros((NPS, 4), np.int64)
        np.add.at(counts, (d_loc, src_bank[m]), 1)
        deg = counts.sum(1)
        order = np.argsort(-deg, kind='stable')
        rem = np.full((T, 4), CAPB, np.int64)
        seats = np.full(T, SEATS, np.int64)
        tile_of = np.full(NPS, -1, np.int64)
        seat_of = np.full(NPS, -1, np.int64)
        for d in order:
            v = counts[d]
            feas = (seats > 0) & (rem >= v).all(1)
            assert feas.any(), "tile packing failed"
            slack = (rem - v).min(1).astype(np.float64)
            slack[~feas] = -1e18
            t = int(np.argmax(slack))
            tile_of[d] = t
            seat_of[d] = SEATS - seats[t]
            rem[t] -= v
            seats[t] -= 1
        tile_of_g[s * NPS:(s + 1) * NPS] = tile_of
        seat_of_g[s * NPS:(s + 1) * NPS] = seat_of

    perm_pos = tile_of_g * P + seat_of_g                  # pos within segment
    perm_row = (np.arange(N) // NPS) * SHR + perm_pos     # global table row

    xf = np.asarray(x, np.float32)
    x_nm = np.zeros((ROWS, F), np.float32)
    x_nm[perm_row] = xf

    indeg = np.bincount(dst, minlength=N).astype(np.float32)
    d_all = np.zeros(ROWS, np.float32)
    d_all[perm_row] = 1.0 + indeg

    in_maps, metas = [], []
    for s in range(SEG):
        lo, hi = s * NPS, (s + 1) * NPS
        g0 = int(batch[lo]); sp = int(batch[hi - 1]) - g0 + 1
        assert sp <= P
        rl, rh = s * SHR, s * SHR + COLS

        brel = np.full(COLS, -1.0, np.float32)
        brel[perm_pos[lo:hi]] = (batch[lo:hi] - g0).astype(np.float32)
        brel_h = np.ascontiguousarray(brel.reshape(T, P).T)          # [128, T]

        filled_row = np.bincount(tile_of_g[lo:hi], minlength=T).astype(np.float32)
        filled_h = np.tile(filled_row, (P, 1)).astype(np.float32)    # [128, T]

        gcnt = np.bincount(batch[lo:hi] - g0, minlength=P).astype(np.float32)
        cnt_h = gcnt.reshape(1, P)

        # edge slots for this core: key = (tile, bank)
        m = seg_of == s
        key = tile_of_g[dst[m]] * 4 + src_bank[m]
        order_e = np.argsort(key, kind='stable')
        key_s = key[order_e]
        cnt_e = np.bincount(key_s, minlength=T * 4)
        assert cnt_e.max() <= CAPB
        cstart = np.zeros(T * 4, np.int64)
        cstart[1:] = np.cumsum(cnt_e)[:-1]
        ne = int(m.sum())
        within = np.arange(ne) - np.repeat(cstart, cnt_e)
        slot = key_s * CAPB + within
        gidx_flat = np.zeros(T * 4 * CAPB, np.int64)
        drel_flat = np.full(T * 4 * CAPB, -1.0, np.float32)
        src_s = src[m][order_e]
        dst_s = dst[m][order_e]
        gidx_flat[slot] = perm_row[src_s] % BANK_ROWS
        drel_flat[slot] = seat_of_g[dst_s]

        # gidx repacked for batched calls: call = (grp, bank) over G tiles
        gi = gidx_flat.reshape(GRP, G, 4, CAPB)
        gi = gi.transpose(0, 2, 1, 3).reshape(GRP * 4, G * CAPB)
        w = gi.reshape(-1, (G * CAPB) // 16, 16).transpose(0, 2, 1)
        w = w.reshape(GRP * 4, 16, (G * CAPB) // 16)
        w = np.concatenate([w[i] for i in range(w.shape[0])], axis=1)
        gidx_h = np.tile(w, (8, 1)).astype(np.int16)     # [128, GRP*4*IDXC]
        drel_h = np.ascontiguousarray(
            drel_flat.reshape(T * 4 * BCPT, P).T)        # [128, T*4*BCPT]

        in_maps.append(dict(
            xsh=x_nm[s * SHR:(s + 1) * SHR].astype(ml_dtypes.bfloat16),
            xfm=np.ascontiguousarray(
                x_nm[rl:rh].T).astype(ml_dtypes.bfloat16),
            gidx=gidx_h, dstrel=drel_h, brel=brel_h, filled=filled_h,
            drow=d_all[rl:rh].reshape(1, COLS),
            cnt=cnt_h,
        ))
        metas.append(dict(g_base=g0, span=sp, gcnt=gcnt))
    return in_maps, metas


def build_kernel(repeat=1, skip_gather=False, skip_select=False,
                 skip_comm=False, light_select=False):
    dt = mybir.dt
    nc = bacc.Bacc("TRN2", target_bir_lowering=False, debug=False,
                   enable_asserts=False, num_devices=NCORES,
                   num_swdge_queues=4)

    xsh_d = nc.dram_tensor("xsh", [SHR, F], dt.bfloat16, kind="ExternalInput")
    xfm_d = nc.dram_tensor("xfm", [P, COLS], dt.bfloat16, kind="ExternalInput")
    gidx_d = nc.dram_tensor("gidx", [P, GRP * 4 * IDXC], dt.int16,
                            kind="ExternalInput")
    drel_d = nc.dram_tensor("dstrel", [P, T * 4 * BCPT], dt.float32,
                            kind="ExternalInput")
    brel_d = nc.dram_tensor("brel", [P, T], dt.float32, kind="ExternalInput")
    drow_d = nc.dram_tensor("drow", [1, COLS], dt.float32, kind="ExternalInput")
    w1_d = nc.dram_tensor("w1", [P, L * F], dt.float32, kind="ExternalInput")
    w2_d = nc.dram_tensor("w2", [P, L * F], dt.float32, kind="ExternalInput")
    bias_d = nc.dram_tensor("bias", [P, 4 * L], dt.float32, kind="ExternalInput")
    iota_d = nc.dram_tensor("iotat", [P, P], dt.float32, kind="ExternalInput")
    ident_d = nc.dram_tensor("ident", [P, P], dt.float32, kind="ExternalInput")
    identb_d = nc.dram_tensor("identb", [P, P], dt.bfloat16,
                              kind="ExternalInput")
    selst_d = nc.dram_tensor("selst", [4 * SEG, 2], dt.bfloat16,
                             kind="ExternalInput")
    pools_d = nc.dram_tensor("pools", [P, L * F], dt.float32,
                             kind="ExternalOutput")
    stats_d = nc.dram_tensor("stats", [P, 2 * L], dt.float32,
                             kind="ExternalOutput")

    # full node tables (AllGather outputs; ping-pong across layers)
    tabs = [nc.dram_tensor(f"tab{i}", [ROWS, F], dt.bfloat16, kind="Internal",
                           addr_space="Shared") for i in range(2)]
    # own-shard bounce buffers (AllGather inputs); last 4 rows carry the
    # layer's BN partial sums as a bf16 hi/lo split of the f32 values
    sh_x = nc.dram_tensor("shx", [SHR, F], dt.bfloat16, kind="Internal")
    shs = [nc.dram_tensor(f"sh{l}", [SHR, F], dt.bfloat16, kind="Internal")
           for l in range(L - 1)]

    inv_n = 1.0 / N
    grp8 = [list(range(NCORES))]

    with tile.TileContext(nc) as tc:
        with tc.tile_pool(name="big", bufs=1) as big, \
             tc.tile_pool(name="gpool", bufs=3) as gpool, \
             tc.tile_pool(name="spool", bufs=12) as spool, \
             tc.tile_pool(name="work", bufs=6) as work, \
             tc.tile_pool(name="stat", bufs=1) as statp, \
             tc.tile_pool(name="psA", bufs=2, space="PSUM") as psA, \
             tc.tile_pool(name="psM", bufs=2, space="PSUM") as psM, \
             tc.tile_pool(name="psT", bufs=2, space="PSUM") as psT, \
             tc.tile_pool(name="psP", bufs=1, space="PSUM") as psP:

            nc.gpsimd.load_library(library_config.mlp)

            gidx_t = big.tile([P, GRP * 4 * IDXC], dt.int16)
            nc.sync.dma_start(gidx_t[:], gidx_d.ap())
            drel_t = big.tile([P, T * 4 * BCPT], dt.float32)
            nc.sync.dma_start(drel_t[:], drel_d.ap())
            brel_t = big.tile([P, T], dt.float32)
            nc.sync.dma_start(brel_t[:], brel_d.ap())
            drow_t = big.tile([1, COLS], dt.float32)
            nc.sync.dma_start(drow_t[:], drow_d.ap())
            selst_t = big.tile([4 * SEG, 2], dt.bfloat16)
            nc.sync.dma_start(selst_t[:], selst_d.ap())
            w1_t = big.tile([P, L * F], dt.float32)
            nc.sync.dma_start(w1_t[:], w1_d.ap())
            w2_t = big.tile([P, L * F], dt.float32)
            nc.sync.dma_start(w2_t[:], w2_d.ap())
            bias_t = big.tile([P, 4 * L], dt.float32)
            nc.sync.dma_start(bias_t[:], bias_d.ap())
            iota_t = big.tile([P, P], dt.float32)
            nc.sync.dma_start(iota_t[:], iota_d.ap())
            ident_t = big.tile([P, P], dt.float32)
            nc.sync.dma_start(ident_t[:], ident_d.ap())
            identb_t = big.tile([P, P], dt.bfloat16)
            nc.sync.dma_start(identb_t[:], identb_d.ap())

            # SBUF-resident feature-major self-term tables (ping-pong)
            hfm = [big.tile([P, COLS], dt.bfloat16, name=f"hfm{i}")
                   for i in range(2)]
            nc.sync.dma_start(hfm[0][:], xfm_d.ap())
            # stage own x shard for the layer-0 AllGather
            nc.sync.dma_start(sh_x.ap(), xsh_d.ap())

            # folded W1 and rank-1 rows for layers 1,2 (layer 0: a=1, c=0)
            w1f_t = big.tile([P, (L - 1) * F], dt.float32)
            w1c_t = big.tile([1, (L - 1) * F], dt.float32)
            ac_t = big.tile([P, 2 * L], dt.float32)
            # phantom-seat z column per layer boundary (p_{-1} = 0); phantom
            # seats flow unmasked through the MLP, their stats contribution
            # (NPH copies of an exactly-reproducible constant column) is
            # subtracted from the BN sums instead of masking every tile.
            pz_t = big.tile([P, 1], dt.float32)
            nc.vector.tensor_scalar(out=pz_t[:], in0=iota_t[:, 0:1],
                                    scalar1=0.0, scalar2=None,
                                    op0=mybir.AluOpType.mult)

            for rep in range(repeat):
              if not skip_comm:
                nc.gpsimd.collective_compute(
                    "AllGather", mybir.AluOpType.bypass, replica_groups=grp8,
                    ins=[sh_x.ap().opt()], outs=[tabs[0].ap().opt()])
              for l in range(L):
                tab = tabs[l % 2]
                hcur = hfm[l % 2]
                hnxt = hfm[(l + 1) % 2]
                b1c = bias_t[:, 0 * L + l:0 * L + l + 1]
                b2c = bias_t[:, 1 * L + l:1 * L + l + 1]
                w2c = w2_t[:, l * F:(l + 1) * F]
                w1c = (w1_t[:, 0:F] if l == 0
                       else w1f_t[:, (l - 1) * F:l * F])

                if l > 0:
                    # decode layer (l-1) global BN stats from the stat rows
                    # that rode the AllGather; fold into this layer's MLP
                    lm = l - 1
                    gac = bias_t[:, 2 * L + lm:2 * L + lm + 1]
                    bec = bias_t[:, 3 * L + lm:3 * L + lm + 1]
                    stg = work.tile([4 * SEG, P], dt.bfloat16, tag="stg",
                                    bufs=2)
                    for cc in range(SEG):
                        nc.sync.dma_start(
                            stg[4 * cc:4 * cc + 4, :],
                            tab.ap()[cc * SHR + COLS:cc * SHR + COLS + 4, :])
                    gps = psM.tile([P, P], dt.float32, tag="mp")
                    nc.tensor.matmul(gps[0:2, :], lhsT=selst_t[:],
                                     rhs=stg[:], start=True, stop=True)
                    gsb = work.tile([2, P], dt.float32, tag="gsb")
                    nc.scalar.copy(gsb[:], gps[0:2, :])
                    gtp = psT.tile([P, P], dt.float32, tag="tt")
                    nc.tensor.matmul(gtp[:, 0:2], lhsT=gsb[:],
                                     rhs=ident_t[0:2, 0:2],
                                     start=True, stop=True)
                    gred = work.tile([P, 2], dt.float32, tag="gred")
                    nc.scalar.copy(gred[:], gtp[:, 0:2])

                    mean = work.tile([P, 1], dt.float32, tag="mean")
                    nc.vector.tensor_scalar(out=mean[:], in0=gred[:, 0:1],
                                            scalar1=inv_n, scalar2=None,
                                            op0=mybir.AluOpType.mult)
                    var = work.tile([P, 1], dt.float32, tag="var")
                    nc.vector.tensor_scalar(out=var[:], in0=gred[:, 1:2],
                                            scalar1=inv_n, scalar2=None,
                                            op0=mybir.AluOpType.mult)
                    msq = work.tile([P, 1], dt.float32, tag="msq")
                    nc.vector.tensor_tensor(out=msq[:], in0=mean[:],
                                            in1=mean[:],
                                            op=mybir.AluOpType.mult)
                    nc.vector.tensor_tensor(out=var[:], in0=var[:],
                                            in1=msq[:],
                                            op=mybir.AluOpType.subtract)
                    nc.vector.tensor_scalar(out=var[:], in0=var[:],
                                            scalar1=BN_EPS, scalar2=None,
                                            op0=mybir.AluOpType.add)
                    sd = work.tile([P, 1], dt.float32, tag="sd")
                    nc.scalar.activation(sd[:], var[:],
                                         mybir.ActivationFunctionType.Sqrt)
                    inv = work.tile([P, 1], dt.float32, tag="inv")
                    nc.vector.reciprocal(inv[:], sd[:])
                    a_c = ac_t[:, 2 * lm:2 * lm + 1]
                    c_c = ac_t[:, 2 * lm + 1:2 * lm + 2]
                    nc.vector.tensor_tensor(out=a_c, in0=inv[:], in1=gac,
                                            op=mybir.AluOpType.mult)
                    tmpc = work.tile([P, 1], dt.float32, tag="tmpc")
                    nc.vector.tensor_tensor(out=tmpc[:], in0=mean[:],
                                            in1=a_c,
                                            op=mybir.AluOpType.mult)
                    nc.vector.tensor_tensor(out=c_c, in0=bec, in1=tmpc[:],
                                            op=mybir.AluOpType.subtract)
                    # W1' = diag(a) W1_l;  w1c_row = (W1_l^T c)^T
                    nc.scalar.activation(
                        w1f_t[:, lm * F:(lm + 1) * F],
                        w1_t[:, l * F:(l + 1) * F],
                        mybir.ActivationFunctionType.Identity,
                        scale=a_c)
                    w1cc = psT.tile([P, P], dt.float32, tag="tt")
                    nc.tensor.matmul(w1cc[:, 0:1],
                                     lhsT=w1_t[:, l * F:(l + 1) * F],
                                     rhs=c_c, start=True, stop=True)
                    w1cs = work.tile([P, 1], dt.float32, tag="w1cs")
                    nc.scalar.copy(w1cs[:], w1cc[:, 0:1])
                    w1cr = psT.tile([P, P], dt.float32, tag="tt")
                    nc.tensor.matmul(w1cr[0:1, :], lhsT=w1cs[:],
                                     rhs=ident_t[:], start=True, stop=True)
                    nc.scalar.copy(w1c_t[:, lm * F:(lm + 1) * F],
                                   w1cr[0:1, :])

                ssum = statp.tile([P, T], dt.float32, tag=f"ssum{l}")
                ssq = statp.tile([P, T], dt.float32, tag=f"ssq{l}")
                pool_ps = psP.tile([P, P], dt.float32, tag="pool")

                for g in range(GRP):
                    g_t = gpool.tile([P, 4, G * BCPT, P], dt.bfloat16, tag="G")
                    call0 = g * 4
                    if skip_gather:
                        # keep the tile allocated; trivial write
                        nc.vector.tensor_copy(g_t[:, 0, 0, 0:2],
                                              iota_t[:, 0:2])
                    if not skip_gather:
                        for b in range(4):
                            nc.gpsimd.dma_gather(
                                out_ap=g_t[:, b],
                                in_ap=tab.ap()[b * BANK_ROWS:
                                               (b + 1) * BANK_ROWS, :],
                                idxs_ap=gidx_t[:, (call0 + b) * IDXC:
                                               (call0 + b + 1) * IDXC],
                                num_idxs=G * CAPB,
                                num_idxs_reg=G * CAPB,
                                elem_size=F,
                                queue_num=b,
                            )
                    for ti in range(G):
                        t = g * G + ti
                        aggT = psA.tile([P, P], dt.float32, tag="agg")
                        s_t = None
                        for b in range(4):
                            for sub in range(BCPT):
                                ch = t * 16 + b * BCPT + sub
                                k = ti * BCPT + sub
                                if skip_select and not (b == 0 and sub == 0):
                                    continue
                                if s_t is None or not light_select:
                                    s_t = spool.tile([P, P], dt.bfloat16,
                                                     tag="S")
                                    nc.vector.tensor_scalar(
                                        out=s_t[:], in0=iota_t[:],
                                        scalar1=drel_t[:, ch:ch + 1],
                                        scalar2=None,
                                        op0=mybir.AluOpType.is_equal)
                                nc.tensor.matmul(
                                    aggT[:], lhsT=g_t[:, b, k, :],
                                    rhs=s_t[:],
                                    start=(b == 0 and sub == 0),
                                    stop=False)
                        # += h_i via identity matmul: z1in lands in PSUM
                        nc.tensor.matmul(
                            aggT[:], lhsT=identb_t[:],
                            rhs=hcur[:, t * P:(t + 1) * P],
                            start=False, stop=True)
                        z1in = work.tile([P, P], dt.float32, tag="z1in")
                        nc.scalar.copy(z1in[:], aggT[:])
                        mp1 = psM.tile([P, P], dt.float32, tag="mp")
                        nc.tensor.matmul(mp1[:], lhsT=w1c, rhs=z1in[:],
                                         start=True, stop=(l == 0))
                        if l > 0:
                            nc.tensor.matmul(
                                mp1[:],
                                lhsT=w1c_t[:, (l - 1) * F:l * F],
                                rhs=drow_t[0:1, t * P:(t + 1) * P],
                                start=False, stop=True)
                        z1 = work.tile([P, P], dt.float32, tag="z1")
                        nc.scalar.activation(
                            z1[:], mp1[:],
                            mybir.ActivationFunctionType.Relu, bias=b1c)
                        mp2 = psM.tile([P, P], dt.float32, tag="mp")
                        nc.tensor.matmul(mp2[:], lhsT=w2c, rhs=z1[:],
                                         start=True, stop=True)
                        zf = work.tile([P, P], dt.float32, tag="zf")
                        nc.scalar.activation(
                            zf[:], mp2[:],
                            mybir.ActivationFunctionType.Relu, bias=b2c)
                        # phantom seats stay unmasked (corrected in stats)
                        nc.vector.tensor_reduce(
                            out=ssum[:, t:t + 1], in_=zf[:],
                            axis=mybir.AxisListType.X,
                            op=mybir.AluOpType.add)
                        sqs = work.tile([P, P], dt.float32, tag="sqs")
                        nc.scalar.activation(
                            sqs[:], zf[:],
                            mybir.ActivationFunctionType.Square,
                            accum_out=ssq[:, t:t + 1])
                        if l < L - 1:
                            nc.scalar.copy(
                                hnxt[:, t * P:(t + 1) * P], zf[:])
                        zT = psT.tile([P, P], dt.float32, tag="tt")
                        nc.tensor.transpose(zT[:], zf[:], ident_t[:])
                        znm = work.tile([P, P], dt.bfloat16, tag="znm")
                        nc.scalar.copy(znm[:], zT[:])
                        if l < L - 1:
                            nc.sync.dma_start(
                                shs[l].ap()[t * P:(t + 1) * P, :], znm[:])
                        sb_t = spool.tile([P, P], dt.bfloat16, tag="S")
                        nc.vector.tensor_scalar(
                            out=sb_t[:], in0=iota_t[:],
                            scalar1=brel_t[:, t:t + 1], scalar2=None,
                            op0=mybir.AluOpType.is_equal)
                        nc.tensor.matmul(pool_ps[:], lhsT=sb_t[:],
                                         rhs=znm[:],
                                         start=(t == 0), stop=(t == T - 1),
                                         skip_group_check=True)

                # raw pool (graph-major) for this layer
                prm = statp.tile([P, P], dt.float32, tag=f"prm{l}")
                nc.scalar.copy(prm[:], pool_ps[:])

                # phantom z column: exactly the per-tile value at a phantom
                # seat, so NPH * pz / NPH * pz^2 corrects the sums. The tile
                # path reads the previous layer's z through bf16 hfm, so
                # round-trip pz through bf16 to match bitwise.
                pzb = work.tile([P, 1], dt.bfloat16, tag="pzb")
                nc.scalar.copy(pzb[:], pz_t[:])
                pzf = work.tile([P, 1], dt.float32, tag="pzf")
                nc.scalar.copy(pzf[:], pzb[:])
                mp1p = psM.tile([P, 1], dt.float32, tag="mpp", bufs=1)
                nc.tensor.matmul(mp1p[:], lhsT=w1c, rhs=pzf[:],
                                 start=True, stop=True)
                z1p = work.tile([P, 1], dt.float32, tag="z1p")
                nc.scalar.activation(z1p[:], mp1p[:],
                                     mybir.ActivationFunctionType.Relu,
                                     bias=b1c)
                mp2p = psM.tile([P, 1], dt.float32, tag="mpp", bufs=1)
                nc.tensor.matmul(mp2p[:], lhsT=w2c, rhs=z1p[:],
                                 start=True, stop=True)
                nc.scalar.activation(pz_t[:], mp2p[:],
                                     mybir.ActivationFunctionType.Relu,
                                     bias=b2c)

                # raw (unnormalized) pool straight to the host
                nc.sync.dma_start(
                    pools_d.ap()[:, l * F:(l + 1) * F], prm[:])

                # ---- local BN partial sums; ship hi/lo split on the AG ----
                NPH = float(T * P - NPS)
                red = work.tile([P, 2], dt.float32, tag="red")
                nc.vector.tensor_reduce(out=red[:, 0:1], in_=ssum[:],
                                        axis=mybir.AxisListType.X,
                                        op=mybir.AluOpType.add)
                nc.vector.tensor_reduce(out=red[:, 1:2], in_=ssq[:],
                                        axis=mybir.AxisListType.X,
                                        op=mybir.AluOpType.add)
                pcor = work.tile([P, 2], dt.float32, tag="pcor")
                nc.vector.tensor_scalar(out=pcor[:, 0:1], in0=pz_t[:],
                                        scalar1=-NPH, scalar2=None,
                                        op0=mybir.AluOpType.mult)
                psq = work.tile([P, 1], dt.float32, tag="psq")
                nc.vector.tensor_tensor(out=psq[:], in0=pz_t[:], in1=pz_t[:],
                                        op=mybir.AluOpType.mult)
                nc.vector.tensor_scalar(out=pcor[:, 1:2], in0=psq[:],
                                        scalar1=-NPH, scalar2=None,
                                        op0=mybir.AluOpType.mult)
                nc.vector.tensor_tensor(out=red[:], in0=red[:], in1=pcor[:],
                                        op=mybir.AluOpType.add)
                nc.sync.dma_start(stats_d.ap()[:, 2 * l:2 * l + 2], red[:])

                if l < L - 1:
                    # red^T as two f32 rows -> bf16 hi/lo rows in the shard
                    redT = psT.tile([P, P], dt.float32, tag="tt")
                    nc.tensor.matmul(redT[0:2, :], lhsT=red[:],
                                     rhs=ident_t[:], start=True, stop=True)
                    redT_sb = work.tile([2, P], dt.float32, tag="redTsb")
                    nc.scalar.copy(redT_sb[:], redT[0:2, :])
                    hi_b = work.tile([2, P], dt.bfloat16, tag="hib")
                    nc.scalar.copy(hi_b[:], redT_sb[:])
                    hi_f = work.tile([2, P], dt.float32, tag="hif")
                    nc.scalar.copy(hi_f[:], hi_b[:])
                    lo_f = work.tile([2, P], dt.float32, tag="lof")
                    nc.vector.tensor_tensor(out=lo_f[:], in0=redT_sb[:],
                                            in1=hi_f[:],
                                            op=mybir.AluOpType.subtract)
                    lo_b = work.tile([2, P], dt.bfloat16, tag="lob")
                    nc.vector.tensor_copy(lo_b[:], lo_f[:])
                    nc.sync.dma_start(
                        shs[l].ap()[COLS:COLS + 2, :], hi_b[:])
                    nc.sync.dma_start(
                        shs[l].ap()[COLS + 2:COLS + 4, :], lo_b[:])
                    if not skip_comm:
                        nc.gpsimd.collective_compute(
                            "AllGather", mybir.AluOpType.bypass,
                            replica_groups=grp8,
                            ins=[shs[l].ap().opt()],
                            outs=[tabs[(l + 1) % 2].ap().opt()])

    nc.compile()
    return nc


def make_in_maps(ins, inputs):
    W1 = np.asarray(inputs['W1'], np.float32)
    W2 = np.asarray(inputs['W2'], np.float32)
    b1 = np.asarray(inputs['b1'], np.float32)
    b2 = np.asarray(inputs['b2'], np.float32)
    gamma = np.asarray(inputs['gamma'], np.float32)
    beta = np.asarray(inputs['beta'], np.float32)
    w1_h = np.ascontiguousarray(np.concatenate([W1[i] for i in range(L)], 1))
    w2_h = np.ascontiguousarray(np.concatenate([W2[i] for i in range(L)], 1))
    bias_h = np.ascontiguousarray(
        np.concatenate([b1.T, b2.T, gamma.T, beta.T], 1))
    iota_h = np.tile(np.arange(P, dtype=np.float32), (P, 1))
    ident_h = np.eye(P, dtype=np.float32)
    pm = np.arange(4 * SEG) % 4
    selst_h = np.stack([((pm == 0) | (pm == 2)), ((pm == 1) | (pm == 3))],
                       axis=1).astype(ml_dtypes.bfloat16)
    shared = {"w1": w1_h, "w2": w2_h, "bias": bias_h,
              "iotat": iota_h, "ident": ident_h,
              "identb": ident_h.astype(ml_dtypes.bfloat16),
              "selst": selst_h}
    return [{**ins[c], **shared} for c in range(NCORES)]


def combine(res, metas, gamma, beta):
    """Global BN stats from per-core partial sums; fix raw pools; combine."""
    gamma = np.asarray(gamma, np.float32)
    beta = np.asarray(beta, np.float32)
    gs = np.zeros((P, 2 * L), np.float64)
    for c in range(NCORES):
        gs += res[c]["stats"].astype(np.float64)
    out = np.zeros((NUM_GRAPHS, L * F), np.float32)
    for l in range(L):
        mean = gs[:, 2 * l] / N
        var = gs[:, 2 * l + 1] / N - mean * mean
        a = gamma[l] / np.sqrt(var + BN_EPS)
        cvec = beta[l] - a * mean
        for c in range(NCORES):
            raw = res[c]["pools"][:, l * F:(l + 1) * F]
            g0, sp = metas[c]['g_base'], metas[c]['span']
            cnt = metas[c]['gcnt'][:sp]
            fixed = a[None, :] * raw[:sp] + np.outer(cnt, cvec)
            out[g0:g0 + sp, l * F:(l + 1) * F] += fixed.astype(np.float32)
    return out


def kernel(x, edge_index, batch, W1, b1, W2, b2, gamma, beta):
    x = np.asarray(x, np.float32)
    edge_index = np.asarray(edge_index, np.int32)
    batch = np.asarray(batch, np.int32)

    ins, metas = preprocess(x, edge_index, batch)
    nc = build_kernel()
    in_maps = make_in_maps(ins, dict(W1=W1, W2=W2, b1=b1, b2=b2,
                                     gamma=gamma, beta=beta))

    import time as _time
    last_exc = None
    for attempt in range(3):
        try:
            res = bass_utils.run_bass_kernel_spmd(
                nc, in_maps, core_ids=list(range(NCORES)))
            break
        except Exception as e:
            last_exc = e
            _time.sleep(20)
    else:
        raise last_exc

    return combine(res.results, metas, gamma, beta)


if __name__ == "__main__":
    import reference
    inputs = reference.setup_inputs()
    inputs = {k: np.asarray(v) for k, v in inputs.items()}
    got = kernel(**inputs)
    print("kernel output shape:", got.shape)


# revision 66
# speedup vs baseline: 1.1589x; 1.0111x over previous
"""GIN encoder (3x GINConv+BN + per-layer global_add_pool) on 8 TRN2 cores.

v3: sharded design. Each core owns one segment of N/8 nodes (104 tiles of
125 seats) and the edges incident to them; per layer it gathers neighbor
rows from a full replicated node table in its DRAM, computes the GIN MLP,
and contributes its shard of the next layer's table via one AllGather
(3.4MB -> 27MB bf16, ~100us). BatchNorm statistics are all-reduced
([128,2] per layer) and folded lazily into the next layer's MLP exactly as
in v2: with h = a*z + c (per-feature), the GIN input
h_i + sum_j h_j = a*(z_i + sum z_j) + c*(1+deg_i), so
  mp1 = (diag(a) W1)^T t + (W1^T c) (x) (1+deg)   [rank-1 via K=1 matmul]
and pools are fixed post-hoc: pool_bn = a*pool_raw + c (x) count_g.
Per-core pools (own segment only) are combined on the host.

The feature-major self-term table and the gather indices live entirely in
SBUF; only the node-major bf16 gather table (written by AllGather) is in
DRAM.
"""
import sys
sys.path.insert(0, '/opt/trn_rl_repo')

import numpy as np
import ml_dtypes

import concourse.bass as bass
import concourse.tile as tile
from concourse import bacc, mybir, library_config
from concourse import bass_utils

NCORES = 8
N = 100000
F = 128
E = 1600000
L = 3
NUM_GRAPHS = 512
BN_EPS = 1e-5
P = 128

SEG = 8                 # segments == cores
NPS = N // SEG          # nodes per segment (12500)
T = 104                 # dst tiles per segment
SEATS = 125             # real seats per tile
CAPB = 512              # slots per (tile, bank)
BCPT = CAPB // 128      # 128-chunks per (tile, bank)
G = 2                   # tiles per gather batch
GRP = T // G            # gather groups per core
IDXC = (G * CAPB) // 16  # gidx cols per call
COLS = T * P            # padded node columns per segment (13312)
SHR = COLS + 4          # shard rows: nodes + 4 bf16 stat rows (hi/lo f32 split)
ROWS = SEG * SHR        # full table rows
NBANK = 4
BANK_ROWS = ROWS // NBANK
assert BANK_ROWS < 32767 and T % G == 0 and T * SEATS >= NPS


def preprocess(x, edge_index, batch):
    src = edge_index[0].astype(np.int64)
    dst = edge_index[1].astype(np.int64)
    seg_of = dst // NPS
    src_bank = src // (2 * NPS)

    tile_of_g = np.empty(N, np.int64)
    seat_of_g = np.empty(N, np.int64)
    for s in range(SEG):
        m = seg_of == s
        d_loc = dst[m] - s * NPS
        counts = np.zeros((NPS, 4), np.int64)
        np.add.at(counts, (d_loc, src_bank[m]), 1)
        deg = counts.sum(1)
        order = np.argsort(-deg, kind='stable')
        rem = np.full((T, 4), CAPB, np.int64)
        seats = np.full(T, SEATS, np.int64)
        tile_of = np.full(NPS, -1, np.int64)
        seat_of = np.full(NPS, -1, np.int64)
        for d in order:
            v = counts[d]
            feas = (seats > 0) & (rem >= v).all(1)
            assert feas.any(), "tile packing failed"
            slack = (rem - v).min(1).astype(np.float64)
            slack[~feas] = -1e18
            t = int(np.argmax(slack))
            tile_of[d] = t
            seat_of[d] = SEATS - seats[t]
            rem[t] -= v
            seats[t] -= 1
        tile_of_g[s * NPS:(s + 1) * NPS] = tile_of
        seat_of_g[s * NPS:(s + 1) * NPS] = seat_of

    perm_pos = tile_of_g * P + seat_of_g                  # pos within segment
    perm_row = (np.arange(N) // NPS) * SHR + perm_pos     # global table row

    xf = np.asarray(x, np.float32)
    x_nm = np.zeros((ROWS, F), np.float32)
    x_nm[perm_row] = xf

    indeg = np.bincount(dst, minlength=N).astype(np.float32)
    d_all = np.zeros(ROWS, np.float32)
    d_all[perm_row] = 1.0 + indeg

    in_maps, metas = [], []
    for s in range(SEG):
        lo, hi = s * NPS, (s + 1) * NPS
        g0 = int(batch[lo]); sp = int(batch[hi - 1]) - g0 + 1
        assert sp <= P
        rl, rh = s * SHR, s * SHR + COLS

        brel = np.full(COLS, -1.0, np.float32)
        brel[perm_pos[lo:hi]] = (batch[lo:hi] - g0).astype(np.float32)
        brel_h = np.ascontiguousarray(brel.reshape(T, P).T)          # [128, T]

        gcnt = np.bincount(batch[lo:hi] - g0, minlength=P).astype(np.float32)
        cnt_h = gcnt.reshape(1, P)

        # edge slots for this core: key = (tile, bank)
        m = seg_of == s
        key = tile_of_g[dst[m]] * 4 + src_bank[m]
        order_e = np.argsort(key, kind='stable')
        key_s = key[order_e]
        cnt_e = np.bincount(key_s, minlength=T * 4)
        assert cnt_e.max() <= CAPB
        cstart = np.zeros(T * 4, np.int64)
        cstart[1:] = np.cumsum(cnt_e)[:-1]
        ne = int(m.sum())
        within = np.arange(ne) - np.repeat(cstart, cnt_e)
        slot = key_s * CAPB + within
        gidx_flat = np.zeros(T * 4 * CAPB, np.int64)
        drel_flat = np.full(T * 4 * CAPB, -1.0, np.float32)
        src_s = src[m][order_e]
        dst_s = dst[m][order_e]
        gidx_flat[slot] = perm_row[src_s] % BANK_ROWS
        drel_flat[slot] = seat_of_g[dst_s]

        # gidx repacked for batched calls: call = (grp, bank) over G tiles
        gi = gidx_flat.reshape(GRP, G, 4, CAPB)
        gi = gi.transpose(0, 2, 1, 3).reshape(GRP * 4, G * CAPB)
        w = gi.reshape(-1, (G * CAPB) // 16, 16).transpose(0, 2, 1)
        w = w.reshape(GRP * 4, 16, (G * CAPB) // 16)
        w = np.concatenate([w[i] for i in range(w.shape[0])], axis=1)
        gidx_h = np.tile(w, (8, 1)).astype(np.int16)     # [128, GRP*4*IDXC]
        drel_h = np.ascontiguousarray(
            drel_flat.reshape(T * 4 * BCPT, P).T)        # [128, T*4*BCPT]

        in_maps.append(dict(
            xsh=x_nm[s * SHR:(s + 1) * SHR].astype(ml_dtypes.bfloat16),
            xfm=np.ascontiguousarray(
                x_nm[rl:rh].T).astype(ml_dtypes.bfloat16),
            gidx=gidx_h, dneg=-drel_h, brel=brel_h,
            drow=d_all[rl:rh].reshape(1, COLS),
            cnt=cnt_h,
        ))
        metas.append(dict(g_base=g0, span=sp, gcnt=gcnt))
    return in_maps, metas


def build_kernel(repeat=1, skip_gather=False, skip_select=False,
                 skip_comm=False, light_select=False):
    dt = mybir.dt
    nc = bacc.Bacc("TRN2", target_bir_lowering=False, debug=False,
                   enable_asserts=False, num_devices=NCORES,
                   num_swdge_queues=4)

    xsh_d = nc.dram_tensor("xsh", [SHR, F], dt.bfloat16, kind="ExternalInput")
    xfm_d = nc.dram_tensor("xfm", [P, COLS], dt.bfloat16, kind="ExternalInput")
    gidx_d = nc.dram_tensor("gidx", [P, GRP * 4 * IDXC], dt.int16,
                            kind="ExternalInput")
    dneg_d = nc.dram_tensor("dneg", [P, T * 4 * BCPT], dt.float32,
                            kind="ExternalInput")
    ineg_d = nc.dram_tensor("ineg", [P, P], dt.float32, kind="ExternalInput")
    brel_d = nc.dram_tensor("brel", [P, T], dt.float32, kind="ExternalInput")
    drow_d = nc.dram_tensor("drow", [1, COLS], dt.float32, kind="ExternalInput")
    w1_d = nc.dram_tensor("w1", [P, L * F], dt.float32, kind="ExternalInput")
    w2_d = nc.dram_tensor("w2", [P, L * F], dt.float32, kind="ExternalInput")
    bias_d = nc.dram_tensor("bias", [P, 4 * L], dt.float32, kind="ExternalInput")
    iota_d = nc.dram_tensor("iotat", [P, P], dt.float32, kind="ExternalInput")
    ident_d = nc.dram_tensor("ident", [P, P], dt.float32, kind="ExternalInput")
    identb_d = nc.dram_tensor("identb", [P, P], dt.bfloat16,
                              kind="ExternalInput")
    selst_d = nc.dram_tensor("selst", [4 * SEG, 2], dt.bfloat16,
                             kind="ExternalInput")
    pools_d = nc.dram_tensor("pools", [P, L * F], dt.float32,
                             kind="ExternalOutput")
    stats_d = nc.dram_tensor("stats", [P, 2 * L], dt.float32,
                             kind="ExternalOutput")

    # full node tables (AllGather outputs; ping-pong across layers)
    tabs = [nc.dram_tensor(f"tab{i}", [ROWS, F], dt.bfloat16, kind="Internal",
                           addr_space="Shared") for i in range(2)]
    # own-shard bounce buffers (AllGather inputs); last 4 rows carry the
    # layer's BN partial sums as a bf16 hi/lo split of the f32 values
    sh_x = nc.dram_tensor("shx", [SHR, F], dt.bfloat16, kind="Internal")
    shs = [nc.dram_tensor(f"sh{l}", [SHR, F], dt.bfloat16, kind="Internal")
           for l in range(L - 1)]

    inv_n = 1.0 / N
    grp8 = [list(range(NCORES))]

    with tile.TileContext(nc) as tc:
        with tc.tile_pool(name="big", bufs=1) as big, \
             tc.tile_pool(name="gpool", bufs=3) as gpool, \
             tc.tile_pool(name="spool", bufs=12) as spool, \
             tc.tile_pool(name="work", bufs=6) as work, \
             tc.tile_pool(name="stat", bufs=1) as statp, \
             tc.tile_pool(name="psA", bufs=2, space="PSUM") as psA, \
             tc.tile_pool(name="psM", bufs=2, space="PSUM") as psM, \
             tc.tile_pool(name="psT", bufs=2, space="PSUM") as psT, \
             tc.tile_pool(name="psP", bufs=1, space="PSUM") as psP:

            nc.gpsimd.load_library(library_config.mlp)

            gidx_t = big.tile([P, GRP * 4 * IDXC], dt.int16)
            nc.sync.dma_start(gidx_t[:], gidx_d.ap())
            dneg_t = big.tile([P, T * 4 * BCPT], dt.float32)
            nc.sync.dma_start(dneg_t[:], dneg_d.ap())
            ineg_t = big.tile([P, P], dt.float32)
            nc.sync.dma_start(ineg_t[:], ineg_d.ap())
            brel_t = big.tile([P, T], dt.float32)
            nc.sync.dma_start(brel_t[:], brel_d.ap())
            drow_t = big.tile([1, COLS], dt.float32)
            nc.sync.dma_start(drow_t[:], drow_d.ap())
            selst_t = big.tile([4 * SEG, 2], dt.bfloat16)
            nc.sync.dma_start(selst_t[:], selst_d.ap())
            w1_t = big.tile([P, L * F], dt.float32)
            nc.sync.dma_start(w1_t[:], w1_d.ap())
            w2_t = big.tile([P, L * F], dt.float32)
            nc.sync.dma_start(w2_t[:], w2_d.ap())
            bias_t = big.tile([P, 4 * L], dt.float32)
            nc.sync.dma_start(bias_t[:], bias_d.ap())
            iota_t = big.tile([P, P], dt.float32)
            nc.sync.dma_start(iota_t[:], iota_d.ap())
            ident_t = big.tile([P, P], dt.float32)
            nc.sync.dma_start(ident_t[:], ident_d.ap())
            identb_t = big.tile([P, P], dt.bfloat16)
            nc.sync.dma_start(identb_t[:], identb_d.ap())

            # SBUF-resident feature-major self-term tables (ping-pong)
            hfm = [big.tile([P, COLS], dt.bfloat16, name=f"hfm{i}")
                   for i in range(2)]
            nc.sync.dma_start(hfm[0][:], xfm_d.ap())
            # stage own x shard for the layer-0 AllGather
            nc.sync.dma_start(sh_x.ap(), xsh_d.ap())

            # folded W1 and rank-1 rows for layers 1,2 (layer 0: a=1, c=0)
            w1f_t = big.tile([P, (L - 1) * F], dt.float32)
            w1c_t = big.tile([1, (L - 1) * F], dt.float32)
            ac_t = big.tile([P, 2 * L], dt.float32)
            # phantom-seat z column per layer boundary (p_{-1} = 0); phantom
            # seats flow unmasked through the MLP, their stats contribution
            # (NPH copies of an exactly-reproducible constant column) is
            # subtracted from the BN sums instead of masking every tile.
            pz_t = big.tile([P, 1], dt.float32)
            nc.vector.tensor_scalar(out=pz_t[:], in0=iota_t[:, 0:1],
                                    scalar1=0.0, scalar2=None,
                                    op0=mybir.AluOpType.mult)

            for rep in range(repeat):
              if not skip_comm:
                nc.gpsimd.collective_compute(
                    "AllGather", mybir.AluOpType.bypass, replica_groups=grp8,
                    ins=[sh_x.ap().opt()], outs=[tabs[0].ap().opt()])
              for l in range(L):
                tab = tabs[l % 2]
                hcur = hfm[l % 2]
                hnxt = hfm[(l + 1) % 2]
                b1c = bias_t[:, 0 * L + l:0 * L + l + 1]
                b2c = bias_t[:, 1 * L + l:1 * L + l + 1]
                w2c = w2_t[:, l * F:(l + 1) * F]
                w1c = (w1_t[:, 0:F] if l == 0
                       else w1f_t[:, (l - 1) * F:l * F])

                if l > 0:
                    # decode layer (l-1) global BN stats from the stat rows
                    # that rode the AllGather; fold into this layer's MLP
                    lm = l - 1
                    gac = bias_t[:, 2 * L + lm:2 * L + lm + 1]
                    bec = bias_t[:, 3 * L + lm:3 * L + lm + 1]
                    stg = work.tile([4 * SEG, P], dt.bfloat16, tag="stg",
                                    bufs=2)
                    for cc in range(SEG):
                        nc.sync.dma_start(
                            stg[4 * cc:4 * cc + 4, :],
                            tab.ap()[cc * SHR + COLS:cc * SHR + COLS + 4, :])
                    gps = psM.tile([P, P], dt.float32, tag="mp")
                    nc.tensor.matmul(gps[0:2, :], lhsT=selst_t[:],
                                     rhs=stg[:], start=True, stop=True)
                    gsb = work.tile([2, P], dt.float32, tag="gsb")
                    nc.scalar.copy(gsb[:], gps[0:2, :])
                    gtp = psT.tile([P, P], dt.float32, tag="tt")
                    nc.tensor.matmul(gtp[:, 0:2], lhsT=gsb[:],
                                     rhs=ident_t[0:2, 0:2],
                                     start=True, stop=True)
                    gred = work.tile([P, 2], dt.float32, tag="gred")
                    nc.scalar.copy(gred[:], gtp[:, 0:2])

                    mean = work.tile([P, 1], dt.float32, tag="mean")
                    nc.vector.tensor_scalar(out=mean[:], in0=gred[:, 0:1],
                                            scalar1=inv_n, scalar2=None,
                                            op0=mybir.AluOpType.mult)
                    var = work.tile([P, 1], dt.float32, tag="var")
                    nc.vector.tensor_scalar(out=var[:], in0=gred[:, 1:2],
                                            scalar1=inv_n, scalar2=None,
                                            op0=mybir.AluOpType.mult)
                    msq = work.tile([P, 1], dt.float32, tag="msq")
                    nc.vector.tensor_tensor(out=msq[:], in0=mean[:],
                                            in1=mean[:],
                                            op=mybir.AluOpType.mult)
                    nc.vector.tensor_tensor(out=var[:], in0=var[:],
                                            in1=msq[:],
                                            op=mybir.AluOpType.subtract)
                    nc.vector.tensor_scalar(out=var[:], in0=var[:],
                                            scalar1=BN_EPS, scalar2=None,
                                            op0=mybir.AluOpType.add)
                    sd = work.tile([P, 1], dt.float32, tag="sd")
                    nc.scalar.activation(sd[:], var[:],
                                         mybir.ActivationFunctionType.Sqrt)
                    inv = work.tile([P, 1], dt.float32, tag="inv")
                    nc.vector.reciprocal(inv[:], sd[:])
                    a_c = ac_t[:, 2 * lm:2 * lm + 1]
                    c_c = ac_t[:, 2 * lm + 1:2 * lm + 2]
                    nc.vector.tensor_tensor(out=a_c, in0=inv[:], in1=gac,
                                            op=mybir.AluOpType.mult)
                    tmpc = work.tile([P, 1], dt.float32, tag="tmpc")
                    nc.vector.tensor_tensor(out=tmpc[:], in0=mean[:],
                                            in1=a_c,
                                            op=mybir.AluOpType.mult)
                    nc.vector.tensor_tensor(out=c_c, in0=bec, in1=tmpc[:],
                                            op=mybir.AluOpType.subtract)
                    # W1' = diag(a) W1_l;  w1c_row = (W1_l^T c)^T
                    nc.scalar.activation(
                        w1f_t[:, lm * F:(lm + 1) * F],
                        w1_t[:, l * F:(l + 1) * F],
                        mybir.ActivationFunctionType.Identity,
                        scale=a_c)
                    w1cc = psT.tile([P, P], dt.float32, tag="tt")
                    nc.tensor.matmul(w1cc[:, 0:1],
                                     lhsT=w1_t[:, l * F:(l + 1) * F],
                                     rhs=c_c, start=True, stop=True)
                    w1cs = work.tile([P, 1], dt.float32, tag="w1cs")
                    nc.scalar.copy(w1cs[:], w1cc[:, 0:1])
                    w1cr = psT.tile([P, P], dt.float32, tag="tt")
                    nc.tensor.matmul(w1cr[0:1, :], lhsT=w1cs[:],
                                     rhs=ident_t[:], start=True, stop=True)
                    nc.scalar.copy(w1c_t[:, lm * F:(lm + 1) * F],
                                   w1cr[0:1, :])

                ssum = statp.tile([P, T], dt.float32, tag=f"ssum{l}")
                ssq = statp.tile([P, T], dt.float32, tag=f"ssq{l}")
                pool_ps = psP.tile([P, P], dt.float32, tag="pool")

                for g in range(GRP):
                    g_t = gpool.tile([P, 4, G * BCPT, P], dt.bfloat16, tag="G")
                    call0 = g * 4
                    if skip_gather:
                        # keep the tile allocated; trivial write
                        nc.vector.tensor_copy(g_t[:, 0, 0, 0:2],
                                              iota_t[:, 0:2])
                    if not skip_gather:
                        for b in range(4):
                            nc.gpsimd.dma_gather(
                                out_ap=g_t[:, b],
                                in_ap=tab.ap()[b * BANK_ROWS:
                                               (b + 1) * BANK_ROWS, :],
                                idxs_ap=gidx_t[:, (call0 + b) * IDXC:
                                               (call0 + b + 1) * IDXC],
                                num_idxs=G * CAPB,
                                num_idxs_reg=G * CAPB,
                                elem_size=F,
                                queue_num=b,
                            )
                    for ti in range(G):
                        t = g * G + ti
                        aggT = psA.tile([P, P], dt.float32, tag="agg")
                        s_t = None
                        for b in range(4):
                            for sub in range(BCPT):
                                ch = t * 16 + b * BCPT + sub
                                k = ti * BCPT + sub
                                if skip_select and not (b == 0 and sub == 0):
                                    continue
                                if s_t is None or not light_select:
                                    s_t = spool.tile([P, P], dt.bfloat16,
                                                     tag="S")
                                    nc.vector.tensor_scalar(
                                        out=s_t[:], in0=ineg_t[:],
                                        scalar1=dneg_t[:, ch:ch + 1],
                                        scalar2=None,
                                        op0=mybir.AluOpType.is_equal)
                                nc.tensor.matmul(
                                    aggT[:], lhsT=g_t[:, b, k, :],
                                    rhs=s_t[:],
                                    start=(b == 0 and sub == 0),
                                    stop=False)
                        # += h_i via identity matmul: z1in lands in PSUM
                        nc.tensor.matmul(
                            aggT[:], lhsT=identb_t[:],
                            rhs=hcur[:, t * P:(t + 1) * P],
                            start=False, stop=True)
                        z1in = work.tile([P, P], dt.float32, tag="z1in")
                        nc.scalar.copy(z1in[:], aggT[:])
                        mp1 = psM.tile([P, P], dt.float32, tag="mp")
                        nc.tensor.matmul(mp1[:], lhsT=w1c, rhs=z1in[:],
                                         start=True, stop=(l == 0))
                        if l > 0:
                            nc.tensor.matmul(
                                mp1[:],
                                lhsT=w1c_t[:, (l - 1) * F:l * F],
                                rhs=drow_t[0:1, t * P:(t + 1) * P],
                                start=False, stop=True)
                        z1 = work.tile([P, P], dt.float32, tag="z1")
                        nc.scalar.activation(
                            z1[:], mp1[:],
                            mybir.ActivationFunctionType.Relu, bias=b1c)
                        mp2 = psM.tile([P, P], dt.float32, tag="mp")
                        nc.tensor.matmul(mp2[:], lhsT=w2c, rhs=z1[:],
                                         start=True, stop=True)
                        zf = work.tile([P, P], dt.float32, tag="zf")
                        nc.scalar.activation(
                            zf[:], mp2[:],
                            mybir.ActivationFunctionType.Relu, bias=b2c)
                        # phantom seats stay unmasked (corrected in stats)
                        nc.vector.tensor_reduce(
                            out=ssum[:, t:t + 1], in_=zf[:],
                            axis=mybir.AxisListType.X,
                            op=mybir.AluOpType.add)
                        sqs = work.tile([P, P], dt.float32, tag="sqs")
                        nc.scalar.activation(
                            sqs[:], zf[:],
                            mybir.ActivationFunctionType.Square,
                            accum_out=ssq[:, t:t + 1])
                        if l < L - 1:
                            nc.scalar.copy(
                                hnxt[:, t * P:(t + 1) * P], zf[:])
                        zT = psT.tile([P, P], dt.float32, tag="tt")
                        nc.tensor.transpose(zT[:], zf[:], ident_t[:])
                        znm = work.tile([P, P], dt.bfloat16, tag="znm")
                        nc.scalar.copy(znm[:], zT[:])
                        if l < L - 1:
                            nc.sync.dma_start(
                                shs[l].ap()[t * P:(t + 1) * P, :], znm[:])
                        sb_t = spool.tile([P, P], dt.bfloat16, tag="S")
                        nc.vector.tensor_scalar(
                            out=sb_t[:], in0=iota_t[:],
                            scalar1=brel_t[:, t:t + 1], scalar2=None,
                            op0=mybir.AluOpType.is_equal)
                        nc.tensor.matmul(pool_ps[:], lhsT=sb_t[:],
                                         rhs=znm[:],
                                         start=(t == 0), stop=(t == T - 1),
                                         skip_group_check=True)

                # raw pool (graph-major) for this layer
                prm = statp.tile([P, P], dt.float32, tag=f"prm{l}")
                nc.scalar.copy(prm[:], pool_ps[:])

                # phantom z column: exactly the per-tile value at a phantom
                # seat, so NPH * pz / NPH * pz^2 corrects the sums. The tile
                # path reads the previous layer's z through bf16 hfm, so
                # round-trip pz through bf16 to match bitwise.
                pzb = work.tile([P, 1], dt.bfloat16, tag="pzb")
                nc.scalar.copy(pzb[:], pz_t[:])
                pzf = work.tile([P, 1], dt.float32, tag="pzf")
                nc.scalar.copy(pzf[:], pzb[:])
                mp1p = psM.tile([P, 1], dt.float32, tag="mpp", bufs=1)
                nc.tensor.matmul(mp1p[:], lhsT=w1c, rhs=pzf[:],
                                 start=True, stop=True)
                z1p = work.tile([P, 1], dt.float32, tag="z1p")
                nc.scalar.activation(z1p[:], mp1p[:],
                                     mybir.ActivationFunctionType.Relu,
                                     bias=b1c)
                mp2p = psM.tile([P, 1], dt.float32, tag="mpp", bufs=1)
                nc.tensor.matmul(mp2p[:], lhsT=w2c, rhs=z1p[:],
                                 start=True, stop=True)
                nc.scalar.activation(pz_t[:], mp2p[:],
                                     mybir.ActivationFunctionType.Relu,
                                     bias=b2c)

                # raw (unnormalized) pool straight to the host
                nc.sync.dma_start(
                    pools_d.ap()[:, l * F:(l + 1) * F], prm[:])

                # ---- local BN partial sums; ship hi/lo split on the AG ----
                NPH = float(T * P - NPS)
                red = work.tile([P, 2], dt.float32, tag="red")
                nc.vector.tensor_reduce(out=red[:, 0:1], in_=ssum[:],
                                        axis=mybir.AxisListType.X,
                                        op=mybir.AluOpType.add)
                nc.vector.tensor_reduce(out=red[:, 1:2], in_=ssq[:],
                                        axis=mybir.AxisListType.X,
                                        op=mybir.AluOpType.add)
                pcor = work.tile([P, 2], dt.float32, tag="pcor")
                nc.vector.tensor_scalar(out=pcor[:, 0:1], in0=pz_t[:],
                                        scalar1=-NPH, scalar2=None,
                                        op0=mybir.AluOpType.mult)
                psq = work.tile([P, 1], dt.float32, tag="psq")
                nc.vector.tensor_tensor(out=psq[:], in0=pz_t[:], in1=pz_t[:],
                                        op=mybir.AluOpType.mult)
                nc.vector.tensor_scalar(out=pcor[:, 1:2], in0=psq[:],
                                        scalar1=-NPH, scalar2=None,
                                        op0=mybir.AluOpType.mult)
                nc.vector.tensor_tensor(out=red[:], in0=red[:], in1=pcor[:],
                                        op=mybir.AluOpType.add)
                nc.sync.dma_start(stats_d.ap()[:, 2 * l:2 * l + 2], red[:])

                if l < L - 1:
                    # red^T as two f32 rows -> bf16 hi/lo rows in the shard
                    redT = psT.tile([P, P], dt.float32, tag="tt")
                    nc.tensor.matmul(redT[0:2, :], lhsT=red[:],
                                     rhs=ident_t[:], start=True, stop=True)
                    redT_sb = work.tile([2, P], dt.float32, tag="redTsb")
                    nc.scalar.copy(redT_sb[:], redT[0:2, :])
                    hi_b = work.tile([2, P], dt.bfloat16, tag="hib")
                    nc.scalar.copy(hi_b[:], redT_sb[:])
                    hi_f = work.tile([2, P], dt.float32, tag="hif")
                    nc.scalar.copy(hi_f[:], hi_b[:])
                    lo_f = work.tile([2, P], dt.float32, tag="lof")
                    nc.vector.tensor_tensor(out=lo_f[:], in0=redT_sb[:],
                                            in1=hi_f[:],
                                            op=mybir.AluOpType.subtract)
                    lo_b = work.tile([2, P], dt.bfloat16, tag="lob")
                    nc.vector.tensor_copy(lo_b[:], lo_f[:])
                    nc.sync.dma_start(
                        shs[l].ap()[COLS:COLS + 2, :], hi_b[:])
                    nc.sync.dma_start(
                        shs[l].ap()[COLS + 2:COLS + 4, :], lo_b[:])
                    if not skip_comm:
                        nc.gpsimd.collective_compute(
                            "AllGather", mybir.AluOpType.bypass,
                            replica_groups=grp8,
                            ins=[shs[l].ap().opt()],
                            outs=[tabs[(l + 1) % 2].ap().opt()])

    nc.compile()
    return nc


def make_in_maps(ins, inputs):
    W1 = np.asarray(inputs['W1'], np.float32)
    W2 = np.asarray(inputs['W2'], np.float32)
    b1 = np.asarray(inputs['b1'], np.float32)
    b2 = np.asarray(inputs['b2'], np.float32)
    gamma = np.asarray(inputs['gamma'], np.float32)
    beta = np.asarray(inputs['beta'], np.float32)
    w1_h = np.ascontiguousarray(np.concatenate([W1[i] for i in range(L)], 1))
    w2_h = np.ascontiguousarray(np.concatenate([W2[i] for i in range(L)], 1))
    bias_h = np.ascontiguousarray(
        np.concatenate([b1.T, b2.T, gamma.T, beta.T], 1))
    iota_h = np.tile(np.arange(P, dtype=np.float32), (P, 1))
    ident_h = np.eye(P, dtype=np.float32)
    pm = np.arange(4 * SEG) % 4
    selst_h = np.stack([((pm == 0) | (pm == 2)), ((pm == 1) | (pm == 3))],
                       axis=1).astype(ml_dtypes.bfloat16)
    shared = {"w1": w1_h, "w2": w2_h, "bias": bias_h,
              "iotat": iota_h, "ineg": -iota_h, "ident": ident_h,
              "identb": ident_h.astype(ml_dtypes.bfloat16),
              "selst": selst_h}
    return [{**ins[c], **shared} for c in range(NCORES)]


def combine(res, metas, gamma, beta):
    """Global BN stats from per-core partial sums; fix raw pools; combine."""
    gamma = np.asarray(gamma, np.float32)
    beta = np.asarray(beta, np.float32)
    gs = np.zeros((P, 2 * L), np.float64)
    for c in range(NCORES):
        gs += res[c]["stats"].astype(np.float64)
    out = np.zeros((NUM_GRAPHS, L * F), np.float32)
    for l in range(L):
        mean = gs[:, 2 * l] / N
        var = gs[:, 2 * l + 1] / N - mean * mean
        a = gamma[l] / np.sqrt(var + BN_EPS)
        cvec = beta[l] - a * mean
        for c in range(NCORES):
            raw = res[c]["pools"][:, l * F:(l + 1) * F]
            g0, sp = metas[c]['g_base'], metas[c]['span']
            cnt = metas[c]['gcnt'][:sp]
            fixed = a[None, :] * raw[:sp] + np.outer(cnt, cvec)
            out[g0:g0 + sp, l * F:(l + 1) * F] += fixed.astype(np.float32)
    return out


def kernel(x, edge_index, batch, W1, b1, W2, b2, gamma, beta):
    x = np.asarray(x, np.float32)
    edge_index = np.asarray(edge_index, np.int32)
    batch = np.asarray(batch, np.int32)

    ins, metas = preprocess(x, edge_index, batch)
    nc = build_kernel()
    in_maps = make_in_maps(ins, dict(W1=W1, W2=W2, b1=b1, b2=b2,
                                     gamma=gamma, beta=beta))

    import time as _time
    last_exc = None
    for attempt in range(3):
        try:
            res = bass_utils.run_bass_kernel_spmd(
                nc, in_maps, core_ids=list(range(NCORES)))
            break
        except Exception as e:
            last_exc = e
            _time.sleep(20)
    else:
        raise last_exc

    return combine(res.results, metas, gamma, beta)


if __name__ == "__main__":
    import reference
    inputs = reference.setup_inputs()
    inputs = {k: np.asarray(v) for k, v in inputs.items()}
    got = kernel(**inputs)
    print("kernel output shape:", got.shape)


# revision 76
# speedup vs baseline: 1.1897x; 1.0265x over previous
"""GIN encoder (3x GINConv+BN + per-layer global_add_pool) on 8 TRN2 cores.

v3: sharded design. Each core owns one segment of N/8 nodes (104 tiles of
125 seats) and the edges incident to them; per layer it gathers neighbor
rows from a full replicated node table in its DRAM, computes the GIN MLP,
and contributes its shard of the next layer's table via one AllGather
(3.4MB -> 27MB bf16, ~100us). BatchNorm statistics are all-reduced
([128,2] per layer) and folded lazily into the next layer's MLP exactly as
in v2: with h = a*z + c (per-feature), the GIN input
h_i + sum_j h_j = a*(z_i + sum z_j) + c*(1+deg_i), so
  mp1 = (diag(a) W1)^T t + (W1^T c) (x) (1+deg)   [rank-1 via K=1 matmul]
and pools are fixed post-hoc: pool_bn = a*pool_raw + c (x) count_g.
Per-core pools (own segment only) are combined on the host.

The feature-major self-term table and the gather indices live entirely in
SBUF; only the node-major bf16 gather table (written by AllGather) is in
DRAM.
"""
import sys
sys.path.insert(0, '/opt/trn_rl_repo')

import numpy as np
import ml_dtypes

import concourse.bass as bass
import concourse.tile as tile
from concourse import bacc, mybir, library_config
from concourse import bass_utils

NCORES = 8
N = 100000
F = 128
E = 1600000
L = 3
NUM_GRAPHS = 512
BN_EPS = 1e-5
P = 128

SEG = 8                 # segments == cores
NPS = N // SEG          # nodes per segment (12500)
T = 104                 # dst tiles per segment
SEATS = 125             # real seats per tile
CAPB = 512              # slots per (tile, bank)
BCPT = CAPB // 128      # 128-chunks per (tile, bank)
G = 2                   # tiles per gather batch
GRP = T // G            # gather groups per core
IDXC = (G * CAPB) // 16  # gidx cols per call
COLS = T * P            # padded node columns per segment (13312)
SHR = COLS + 4          # shard rows: nodes + 4 bf16 stat rows (hi/lo f32 split)
ROWS = SEG * SHR        # full table rows
NBANK = 4
BANK_ROWS = ROWS // NBANK
assert BANK_ROWS < 32767 and T % G == 0 and T * SEATS >= NPS


def preprocess(x, edge_index, batch):
    src = edge_index[0].astype(np.int64)
    dst = edge_index[1].astype(np.int64)
    seg_of = dst // NPS
    src_bank = src // (2 * NPS)

    tile_of_g = np.empty(N, np.int64)
    seat_of_g = np.empty(N, np.int64)
    for s in range(SEG):
        m = seg_of == s
        d_loc = dst[m] - s * NPS
        counts = np.zeros((NPS, 4), np.int64)
        np.add.at(counts, (d_loc, src_bank[m]), 1)
        deg = counts.sum(1)
        order = np.argsort(-deg, kind='stable')
        rem = np.full((T, 4), CAPB, np.int64)
        seats = np.full(T, SEATS, np.int64)
        tile_of = np.full(NPS, -1, np.int64)
        seat_of = np.full(NPS, -1, np.int64)
        for d in order:
            v = counts[d]
            feas = (seats > 0) & (rem >= v).all(1)
            assert feas.any(), "tile packing failed"
            slack = (rem - v).min(1).astype(np.float64)
            slack[~feas] = -1e18
            t = int(np.argmax(slack))
            tile_of[d] = t
            seat_of[d] = SEATS - seats[t]
            rem[t] -= v
            seats[t] -= 1
        tile_of_g[s * NPS:(s + 1) * NPS] = tile_of
        seat_of_g[s * NPS:(s + 1) * NPS] = seat_of

    perm_pos = tile_of_g * P + seat_of_g                  # pos within segment
    perm_row = (np.arange(N) // NPS) * SHR + perm_pos     # global table row

    xf = np.asarray(x, np.float32)
    x_nm = np.zeros((ROWS, F), np.float32)
    x_nm[perm_row] = xf

    indeg = np.bincount(dst, minlength=N).astype(np.float32)
    d_all = np.zeros(ROWS, np.float32)
    d_all[perm_row] = 1.0 + indeg

    in_maps, metas = [], []
    for s in range(SEG):
        lo, hi = s * NPS, (s + 1) * NPS
        g0 = int(batch[lo]); sp = int(batch[hi - 1]) - g0 + 1
        assert sp <= P
        rl, rh = s * SHR, s * SHR + COLS

        brel = np.full(COLS, -1.0, np.float32)
        brel[perm_pos[lo:hi]] = (batch[lo:hi] - g0).astype(np.float32)
        brel_h = np.ascontiguousarray(brel.reshape(T, P).T)          # [128, T]

        gcnt = np.bincount(batch[lo:hi] - g0, minlength=P).astype(np.float32)
        cnt_h = gcnt.reshape(1, P)

        # edge slots for this core: key = (tile, bank)
        m = seg_of == s
        key = tile_of_g[dst[m]] * 4 + src_bank[m]
        order_e = np.argsort(key, kind='stable')
        key_s = key[order_e]
        cnt_e = np.bincount(key_s, minlength=T * 4)
        assert cnt_e.max() <= CAPB
        cstart = np.zeros(T * 4, np.int64)
        cstart[1:] = np.cumsum(cnt_e)[:-1]
        ne = int(m.sum())
        within = np.arange(ne) - np.repeat(cstart, cnt_e)
        slot = key_s * CAPB + within
        gidx_flat = np.zeros(T * 4 * CAPB, np.int64)
        drel_flat = np.full(T * 4 * CAPB, -1.0, np.float32)
        src_s = src[m][order_e]
        dst_s = dst[m][order_e]
        gidx_flat[slot] = perm_row[src_s] % BANK_ROWS
        drel_flat[slot] = seat_of_g[dst_s]

        # gidx repacked for batched calls: call = (grp, bank) over G tiles
        gi = gidx_flat.reshape(GRP, G, 4, CAPB)
        gi = gi.transpose(0, 2, 1, 3).reshape(GRP * 4, G * CAPB)
        w = gi.reshape(-1, (G * CAPB) // 16, 16).transpose(0, 2, 1)
        w = w.reshape(GRP * 4, 16, (G * CAPB) // 16)
        w = np.concatenate([w[i] for i in range(w.shape[0])], axis=1)
        gidx_h = np.tile(w, (8, 1)).astype(np.int16)     # [128, GRP*4*IDXC]
        drel_h = np.ascontiguousarray(
            drel_flat.reshape(T * 4 * BCPT, P).T)        # [128, T*4*BCPT]

        in_maps.append(dict(
            xsh=x_nm[s * SHR:(s + 1) * SHR].astype(ml_dtypes.bfloat16),
            xfm=np.ascontiguousarray(
                x_nm[rl:rh].T).astype(ml_dtypes.bfloat16),
            gidx=gidx_h, dneg=-drel_h, brel=brel_h,
            drow=d_all[rl:rh].reshape(1, COLS),
            cnt=cnt_h,
        ))
        metas.append(dict(g_base=g0, span=sp, gcnt=gcnt))
    return in_maps, metas


def build_kernel(repeat=1, skip_gather=False, skip_select=False,
                 skip_comm=False, light_select=False, fat_desc=False):
    dt = mybir.dt
    nc = bacc.Bacc("TRN2", target_bir_lowering=False, debug=False,
                   enable_asserts=False, num_devices=NCORES,
                   num_swdge_queues=4)

    xsh_d = nc.dram_tensor("xsh", [SHR, F], dt.bfloat16, kind="ExternalInput")
    xfm_d = nc.dram_tensor("xfm", [P, COLS], dt.bfloat16, kind="ExternalInput")
    gidx_d = nc.dram_tensor("gidx", [P, GRP * 4 * IDXC], dt.int16,
                            kind="ExternalInput")
    dneg_d = nc.dram_tensor("dneg", [P, T * 4 * BCPT], dt.float32,
                            kind="ExternalInput")
    ineg_d = nc.dram_tensor("ineg", [P, P], dt.float32, kind="ExternalInput")
    brel_d = nc.dram_tensor("brel", [P, T], dt.float32, kind="ExternalInput")
    drow_d = nc.dram_tensor("drow", [1, COLS], dt.float32, kind="ExternalInput")
    w1_d = nc.dram_tensor("w1", [P, L * F], dt.float32, kind="ExternalInput")
    w2_d = nc.dram_tensor("w2", [P, L * F], dt.float32, kind="ExternalInput")
    bias_d = nc.dram_tensor("bias", [P, 4 * L], dt.float32, kind="ExternalInput")
    iota_d = nc.dram_tensor("iotat", [P, P], dt.float32, kind="ExternalInput")
    ident_d = nc.dram_tensor("ident", [P, P], dt.float32, kind="ExternalInput")
    identb_d = nc.dram_tensor("identb", [P, P], dt.bfloat16,
                              kind="ExternalInput")
    selst_d = nc.dram_tensor("selst", [4 * SEG, 2], dt.bfloat16,
                             kind="ExternalInput")
    pools_d = nc.dram_tensor("pools", [P, L * F], dt.float32,
                             kind="ExternalOutput")
    stats_d = nc.dram_tensor("stats", [P, 2 * L], dt.float32,
                             kind="ExternalOutput")

    # full node tables (AllGather outputs; ping-pong across layers)
    tabs = [nc.dram_tensor(f"tab{i}", [ROWS, F], dt.bfloat16, kind="Internal",
                           addr_space="Shared") for i in range(2)]
    # own-shard bounce buffers (AllGather inputs); last 4 rows carry the
    # layer's BN partial sums as a bf16 hi/lo split of the f32 values
    tabfat = (nc.dram_tensor("tabfat", [ROWS // 2, 2 * F], dt.bfloat16,
                             kind="Internal") if fat_desc else None)
    sh_x = nc.dram_tensor("shx", [SHR, F], dt.bfloat16, kind="Internal")
    shs = [nc.dram_tensor(f"sh{l}", [SHR, F], dt.bfloat16, kind="Internal")
           for l in range(L - 1)]

    inv_n = 1.0 / N
    grp8 = [list(range(NCORES))]

    with tile.TileContext(nc) as tc:
        with tc.tile_pool(name="big", bufs=1) as big, \
             tc.tile_pool(name="gpool", bufs=3) as gpool, \
             tc.tile_pool(name="spool", bufs=24) as spool, \
             tc.tile_pool(name="work", bufs=5) as work, \
             tc.tile_pool(name="stat", bufs=1) as statp, \
             tc.tile_pool(name="psA", bufs=2, space="PSUM") as psA, \
             tc.tile_pool(name="psM", bufs=2, space="PSUM") as psM, \
             tc.tile_pool(name="psT", bufs=2, space="PSUM") as psT, \
             tc.tile_pool(name="psP", bufs=1, space="PSUM") as psP:

            nc.gpsimd.load_library(library_config.mlp)

            gidx_t = big.tile([P, GRP * 4 * IDXC], dt.int16)
            nc.sync.dma_start(gidx_t[:], gidx_d.ap())
            dneg_t = big.tile([P, T * 4 * BCPT], dt.float32)
            nc.sync.dma_start(dneg_t[:], dneg_d.ap())
            ineg_t = big.tile([P, P], dt.float32)
            nc.sync.dma_start(ineg_t[:], ineg_d.ap())
            brel_t = big.tile([P, T], dt.float32)
            nc.sync.dma_start(brel_t[:], brel_d.ap())
            drow_t = big.tile([1, COLS], dt.float32)
            nc.sync.dma_start(drow_t[:], drow_d.ap())
            selst_t = big.tile([4 * SEG, 2], dt.bfloat16)
            nc.sync.dma_start(selst_t[:], selst_d.ap())
            w1_t = big.tile([P, L * F], dt.float32)
            nc.sync.dma_start(w1_t[:], w1_d.ap())
            w2_t = big.tile([P, L * F], dt.float32)
            nc.sync.dma_start(w2_t[:], w2_d.ap())
            bias_t = big.tile([P, 4 * L], dt.float32)
            nc.sync.dma_start(bias_t[:], bias_d.ap())
            iota_t = big.tile([P, P], dt.float32)
            nc.sync.dma_start(iota_t[:], iota_d.ap())
            ident_t = big.tile([P, P], dt.float32)
            nc.sync.dma_start(ident_t[:], ident_d.ap())
            identb_t = big.tile([P, P], dt.bfloat16)
            nc.sync.dma_start(identb_t[:], identb_d.ap())

            # SBUF-resident feature-major self-term tables (ping-pong)
            hfm = [big.tile([P, COLS], dt.bfloat16, name=f"hfm{i}")
                   for i in range(2)]
            nc.sync.dma_start(hfm[0][:], xfm_d.ap())
            # stage own x shard for the layer-0 AllGather
            nc.sync.dma_start(sh_x.ap(), xsh_d.ap())

            # folded W1 and rank-1 rows for layers 1,2 (layer 0: a=1, c=0)
            w1f_t = big.tile([P, (L - 1) * F], dt.float32)
            w1c_t = big.tile([1, (L - 1) * F], dt.float32)
            ac_t = big.tile([P, 2 * L], dt.float32)
            # phantom-seat z column per layer boundary (p_{-1} = 0); phantom
            # seats flow unmasked through the MLP, their stats contribution
            # (NPH copies of an exactly-reproducible constant column) is
            # subtracted from the BN sums instead of masking every tile.
            pz_t = big.tile([P, 1], dt.float32)
            nc.vector.tensor_scalar(out=pz_t[:], in0=iota_t[:, 0:1],
                                    scalar1=0.0, scalar2=None,
                                    op0=mybir.AluOpType.mult)

            for rep in range(repeat):
              if not skip_comm:
                nc.gpsimd.collective_compute(
                    "AllGather", mybir.AluOpType.bypass, replica_groups=grp8,
                    ins=[sh_x.ap().opt()], outs=[tabs[0].ap().opt()])
              for l in range(L):
                tab = tabs[l % 2]
                hcur = hfm[l % 2]
                hnxt = hfm[(l + 1) % 2]
                b1c = bias_t[:, 0 * L + l:0 * L + l + 1]
                b2c = bias_t[:, 1 * L + l:1 * L + l + 1]
                w2c = w2_t[:, l * F:(l + 1) * F]
                w1c = (w1_t[:, 0:F] if l == 0
                       else w1f_t[:, (l - 1) * F:l * F])

                if l > 0:
                    # decode layer (l-1) global BN stats from the stat rows
                    # that rode the AllGather; fold into this layer's MLP
                    lm = l - 1
                    gac = bias_t[:, 2 * L + lm:2 * L + lm + 1]
                    bec = bias_t[:, 3 * L + lm:3 * L + lm + 1]
                    stg = work.tile([4 * SEG, P], dt.bfloat16, tag="stg",
                                    bufs=2)
                    for cc in range(SEG):
                        nc.sync.dma_start(
                            stg[4 * cc:4 * cc + 4, :],
                            tab.ap()[cc * SHR + COLS:cc * SHR + COLS + 4, :])
                    gps = psM.tile([P, P], dt.float32, tag="mp")
                    nc.tensor.matmul(gps[0:2, :], lhsT=selst_t[:],
                                     rhs=stg[:], start=True, stop=True)
                    gsb = work.tile([2, P], dt.float32, tag="gsb")
                    nc.scalar.copy(gsb[:], gps[0:2, :])
                    gtp = psT.tile([P, P], dt.float32, tag="tt")
                    nc.tensor.matmul(gtp[:, 0:2], lhsT=gsb[:],
                                     rhs=ident_t[0:2, 0:2],
                                     start=True, stop=True)
                    gred = work.tile([P, 2], dt.float32, tag="gred")
                    nc.scalar.copy(gred[:], gtp[:, 0:2])

                    mean = work.tile([P, 1], dt.float32, tag="mean")
                    nc.vector.tensor_scalar(out=mean[:], in0=gred[:, 0:1],
                                            scalar1=inv_n, scalar2=None,
                                            op0=mybir.AluOpType.mult)
                    var = work.tile([P, 1], dt.float32, tag="var")
                    nc.vector.tensor_scalar(out=var[:], in0=gred[:, 1:2],
                                            scalar1=inv_n, scalar2=None,
                                            op0=mybir.AluOpType.mult)
                    msq = work.tile([P, 1], dt.float32, tag="msq")
                    nc.vector.tensor_tensor(out=msq[:], in0=mean[:],
                                            in1=mean[:],
                                            op=mybir.AluOpType.mult)
                    nc.vector.tensor_tensor(out=var[:], in0=var[:],
                                            in1=msq[:],
                                            op=mybir.AluOpType.subtract)
                    nc.vector.tensor_scalar(out=var[:], in0=var[:],
                                            scalar1=BN_EPS, scalar2=None,
                                            op0=mybir.AluOpType.add)
                    sd = work.tile([P, 1], dt.float32, tag="sd")
                    nc.scalar.activation(sd[:], var[:],
                                         mybir.ActivationFunctionType.Sqrt)
                    inv = work.tile([P, 1], dt.float32, tag="inv")
                    nc.vector.reciprocal(inv[:], sd[:])
                    a_c = ac_t[:, 2 * lm:2 * lm + 1]
                    c_c = ac_t[:, 2 * lm + 1:2 * lm + 2]
                    nc.vector.tensor_tensor(out=a_c, in0=inv[:], in1=gac,
                                            op=mybir.AluOpType.mult)
                    tmpc = work.tile([P, 1], dt.float32, tag="tmpc")
                    nc.vector.tensor_tensor(out=tmpc[:], in0=mean[:],
                                            in1=a_c,
                                            op=mybir.AluOpType.mult)
                    nc.vector.tensor_tensor(out=c_c, in0=bec, in1=tmpc[:],
                                            op=mybir.AluOpType.subtract)
                    # W1' = diag(a) W1_l;  w1c_row = (W1_l^T c)^T
                    nc.scalar.activation(
                        w1f_t[:, lm * F:(lm + 1) * F],
                        w1_t[:, l * F:(l + 1) * F],
                        mybir.ActivationFunctionType.Identity,
                        scale=a_c)
                    w1cc = psT.tile([P, P], dt.float32, tag="tt")
                    nc.tensor.matmul(w1cc[:, 0:1],
                                     lhsT=w1_t[:, l * F:(l + 1) * F],
                                     rhs=c_c, start=True, stop=True)
                    w1cs = work.tile([P, 1], dt.float32, tag="w1cs")
                    nc.scalar.copy(w1cs[:], w1cc[:, 0:1])
                    w1cr = psT.tile([P, P], dt.float32, tag="tt")
                    nc.tensor.matmul(w1cr[0:1, :], lhsT=w1cs[:],
                                     rhs=ident_t[:], start=True, stop=True)
                    nc.scalar.copy(w1c_t[:, lm * F:(lm + 1) * F],
                                   w1cr[0:1, :])

                ssum = statp.tile([P, T], dt.float32, tag=f"ssum{l}")
                ssq = statp.tile([P, T], dt.float32, tag=f"ssq{l}")
                pool_ps = psP.tile([P, P], dt.float32, tag="pool")

                for g in range(GRP):
                    if fat_desc:
                        g_t = gpool.tile([P, 4, (G * BCPT) // 2, 2 * P],
                                         dt.bfloat16, tag="G")
                    else:
                        g_t = gpool.tile([P, 4, G * BCPT, P], dt.bfloat16,
                                         tag="G")
                    call0 = g * 4
                    if skip_gather:
                        # keep the tile allocated; trivial write
                        nc.vector.tensor_copy(g_t[:, 0, 0, 0:2],
                                              iota_t[:, 0:2])
                    if not skip_gather:
                        for b in range(4):
                            if fat_desc:
                                # TIMING DIAGNOSTIC ONLY: half the descs at
                                # 2x size (wrong data, in-bounds reads)
                                nc.gpsimd.dma_gather(
                                    out_ap=g_t[:, b],
                                    in_ap=tabfat.ap()[0:BANK_ROWS // 2, :],
                                    idxs_ap=gidx_t[:, (call0 + b) * IDXC:
                                                   (call0 + b) * IDXC
                                                   + IDXC // 2],
                                    num_idxs=(G * CAPB) // 2,
                                    num_idxs_reg=(G * CAPB) // 2,
                                    elem_size=2 * F,
                                    queue_num=b,
                                )
                                continue
                            nc.gpsimd.dma_gather(
                                out_ap=g_t[:, b],
                                in_ap=tab.ap()[b * BANK_ROWS:
                                               (b + 1) * BANK_ROWS, :],
                                idxs_ap=gidx_t[:, (call0 + b) * IDXC:
                                               (call0 + b + 1) * IDXC],
                                num_idxs=G * CAPB,
                                num_idxs_reg=G * CAPB,
                                elem_size=F,
                                queue_num=b,
                            )
                    for ti in range(G):
                        t = g * G + ti
                        aggT = psA.tile([P, P], dt.float32, tag="agg")
                        s_t = None
                        for b in range(4):
                            for sub in range(BCPT):
                                ch = t * 16 + b * BCPT + sub
                                k = ti * BCPT + sub
                                if skip_select and not (b == 0 and sub == 0):
                                    continue
                                if s_t is None or not light_select:
                                    s_t = spool.tile([P, P], dt.bfloat16,
                                                     tag="S")
                                    nc.vector.tensor_scalar(
                                        out=s_t[:], in0=ineg_t[:],
                                        scalar1=dneg_t[:, ch:ch + 1],
                                        scalar2=None,
                                        op0=mybir.AluOpType.is_equal)
                                glhs = (g_t[:, b, k // 2,
                                            (k % 2) * P:(k % 2 + 1) * P]
                                        if fat_desc else g_t[:, b, k, :])
                                nc.tensor.matmul(
                                    aggT[:], lhsT=glhs,
                                    rhs=s_t[:],
                                    start=(b == 0 and sub == 0),
                                    stop=False)
                        # += h_i via identity matmul: z1in lands in PSUM
                        nc.tensor.matmul(
                            aggT[:], lhsT=identb_t[:],
                            rhs=hcur[:, t * P:(t + 1) * P],
                            start=False, stop=True)
                        z1in = work.tile([P, P], dt.float32, tag="z1in")
                        nc.scalar.copy(z1in[:], aggT[:])
                        mp1 = psM.tile([P, P], dt.float32, tag="mp")
                        nc.tensor.matmul(mp1[:], lhsT=w1c, rhs=z1in[:],
                                         start=True, stop=(l == 0))
                        if l > 0:
                            nc.tensor.matmul(
                                mp1[:],
                                lhsT=w1c_t[:, (l - 1) * F:l * F],
                                rhs=drow_t[0:1, t * P:(t + 1) * P],
                                start=False, stop=True)
                        z1 = work.tile([P, P], dt.float32, tag="z1")
                        nc.scalar.activation(
                            z1[:], mp1[:],
                            mybir.ActivationFunctionType.Relu, bias=b1c)
                        mp2 = psM.tile([P, P], dt.float32, tag="mp")
                        nc.tensor.matmul(mp2[:], lhsT=w2c, rhs=z1[:],
                                         start=True, stop=True)
                        zf = work.tile([P, P], dt.float32, tag="zf")
                        nc.scalar.activation(
                            zf[:], mp2[:],
                            mybir.ActivationFunctionType.Relu, bias=b2c)
                        # phantom seats stay unmasked (corrected in stats).
                        # ssum rides the hfm copy's accum_out so the DVE
                        # queue holds only dependency-free selects (no
                        # head-of-line stall on zf).
                        sqs = work.tile([P, P], dt.bfloat16, tag="sqs")
                        nc.scalar.activation(
                            sqs[:], zf[:],
                            mybir.ActivationFunctionType.Square,
                            accum_out=ssq[:, t:t + 1])
                        nc.scalar.activation(
                            hnxt[:, t * P:(t + 1) * P], zf[:],
                            mybir.ActivationFunctionType.Identity,
                            accum_out=ssum[:, t:t + 1])
                        zT = psT.tile([P, P], dt.float32, tag="tt")
                        nc.tensor.transpose(zT[:], zf[:], ident_t[:])
                        znm = work.tile([P, P], dt.bfloat16, tag="znm")
                        nc.scalar.copy(znm[:], zT[:])
                        if l < L - 1:
                            nc.sync.dma_start(
                                shs[l].ap()[t * P:(t + 1) * P, :], znm[:])
                        sb_t = spool.tile([P, P], dt.bfloat16, tag="S")
                        nc.vector.tensor_scalar(
                            out=sb_t[:], in0=iota_t[:],
                            scalar1=brel_t[:, t:t + 1], scalar2=None,
                            op0=mybir.AluOpType.is_equal)
                        nc.tensor.matmul(pool_ps[:], lhsT=sb_t[:],
                                         rhs=znm[:],
                                         start=(t == 0), stop=(t == T - 1),
                                         skip_group_check=True)

                # raw pool (graph-major) for this layer
                prm = statp.tile([P, P], dt.float32, tag=f"prm{l}")
                nc.scalar.copy(prm[:], pool_ps[:])

                # phantom z column: exactly the per-tile value at a phantom
                # seat, so NPH * pz / NPH * pz^2 corrects the sums. The tile
                # path reads the previous layer's z through bf16 hfm, so
                # round-trip pz through bf16 to match bitwise.
                pzb = work.tile([P, 1], dt.bfloat16, tag="pzb")
                nc.scalar.copy(pzb[:], pz_t[:])
                pzf = work.tile([P, 1], dt.float32, tag="pzf")
                nc.scalar.copy(pzf[:], pzb[:])
                mp1p = psM.tile([P, 1], dt.float32, tag="mpp", bufs=1)
                nc.tensor.matmul(mp1p[:], lhsT=w1c, rhs=pzf[:],
                                 start=True, stop=True)
                z1p = work.tile([P, 1], dt.float32, tag="z1p")
                nc.scalar.activation(z1p[:], mp1p[:],
                                     mybir.ActivationFunctionType.Relu,
                                     bias=b1c)
                mp2p = psM.tile([P, 1], dt.float32, tag="mpp", bufs=1)
                nc.tensor.matmul(mp2p[:], lhsT=w2c, rhs=z1p[:],
                                 start=True, stop=True)
                nc.scalar.activation(pz_t[:], mp2p[:],
                                     mybir.ActivationFunctionType.Relu,
                                     bias=b2c)

                # raw (unnormalized) pool straight to the host
                nc.sync.dma_start(
                    pools_d.ap()[:, l * F:(l + 1) * F], prm[:])

                # ---- local BN partial sums; ship hi/lo split on the AG ----
                NPH = float(T * P - NPS)
                red = work.tile([P, 2], dt.float32, tag="red")
                nc.vector.tensor_reduce(out=red[:, 0:1], in_=ssum[:],
                                        axis=mybir.AxisListType.X,
                                        op=mybir.AluOpType.add)
                nc.vector.tensor_reduce(out=red[:, 1:2], in_=ssq[:],
                                        axis=mybir.AxisListType.X,
                                        op=mybir.AluOpType.add)
                pcor = work.tile([P, 2], dt.float32, tag="pcor")
                nc.vector.tensor_scalar(out=pcor[:, 0:1], in0=pz_t[:],
                                        scalar1=-NPH, scalar2=None,
                                        op0=mybir.AluOpType.mult)
                psq = work.tile([P, 1], dt.float32, tag="psq")
                nc.vector.tensor_tensor(out=psq[:], in0=pz_t[:], in1=pz_t[:],
                                        op=mybir.AluOpType.mult)
                nc.vector.tensor_scalar(out=pcor[:, 1:2], in0=psq[:],
                                        scalar1=-NPH, scalar2=None,
                                        op0=mybir.AluOpType.mult)
                nc.vector.tensor_tensor(out=red[:], in0=red[:], in1=pcor[:],
                                        op=mybir.AluOpType.add)
                nc.sync.dma_start(stats_d.ap()[:, 2 * l:2 * l + 2], red[:])

                if l < L - 1:
                    # red^T as two f32 rows -> bf16 hi/lo rows in the shard
                    redT = psT.tile([P, P], dt.float32, tag="tt")
                    nc.tensor.matmul(redT[0:2, :], lhsT=red[:],
                                     rhs=ident_t[:], start=True, stop=True)
                    redT_sb = work.tile([2, P], dt.float32, tag="redTsb")
                    nc.scalar.copy(redT_sb[:], redT[0:2, :])
                    hi_b = work.tile([2, P], dt.bfloat16, tag="hib")
                    nc.scalar.copy(hi_b[:], redT_sb[:])
                    hi_f = work.tile([2, P], dt.float32, tag="hif")
                    nc.scalar.copy(hi_f[:], hi_b[:])
                    lo_f = work.tile([2, P], dt.float32, tag="lof")
                    nc.vector.tensor_tensor(out=lo_f[:], in0=redT_sb[:],
                                            in1=hi_f[:],
                                            op=mybir.AluOpType.subtract)
                    lo_b = work.tile([2, P], dt.bfloat16, tag="lob")
                    nc.vector.tensor_copy(lo_b[:], lo_f[:])
                    nc.sync.dma_start(
                        shs[l].ap()[COLS:COLS + 2, :], hi_b[:])
                    nc.sync.dma_start(
                        shs[l].ap()[COLS + 2:COLS + 4, :], lo_b[:])
                    if not skip_comm:
                        nc.gpsimd.collective_compute(
                            "AllGather", mybir.AluOpType.bypass,
                            replica_groups=grp8,
                            ins=[shs[l].ap().opt()],
                            outs=[tabs[(l + 1) % 2].ap().opt()])

    nc.compile()
    return nc


def make_in_maps(ins, inputs):
    W1 = np.asarray(inputs['W1'], np.float32)
    W2 = np.asarray(inputs['W2'], np.float32)
    b1 = np.asarray(inputs['b1'], np.float32)
    b2 = np.asarray(inputs['b2'], np.float32)
    gamma = np.asarray(inputs['gamma'], np.float32)
    beta = np.asarray(inputs['beta'], np.float32)
    w1_h = np.ascontiguousarray(np.concatenate([W1[i] for i in range(L)], 1))
    w2_h = np.ascontiguousarray(np.concatenate([W2[i] for i in range(L)], 1))
    bias_h = np.ascontiguousarray(
        np.concatenate([b1.T, b2.T, gamma.T, beta.T], 1))
    iota_h = np.tile(np.arange(P, dtype=np.float32), (P, 1))
    ident_h = np.eye(P, dtype=np.float32)
    pm = np.arange(4 * SEG) % 4
    selst_h = np.stack([((pm == 0) | (pm == 2)), ((pm == 1) | (pm == 3))],
                       axis=1).astype(ml_dtypes.bfloat16)
    shared = {"w1": w1_h, "w2": w2_h, "bias": bias_h,
              "iotat": iota_h, "ineg": -iota_h, "ident": ident_h,
              "identb": ident_h.astype(ml_dtypes.bfloat16),
              "selst": selst_h}
    return [{**ins[c], **shared} for c in range(NCORES)]


def combine(res, metas, gamma, beta):
    """Global BN stats from per-core partial sums; fix raw pools; combine."""
    gamma = np.asarray(gamma, np.float32)
    beta = np.asarray(beta, np.float32)
    gs = np.zeros((P, 2 * L), np.float64)
    for c in range(NCORES):
        gs += res[c]["stats"].astype(np.float64)
    out = np.zeros((NUM_GRAPHS, L * F), np.float32)
    for l in range(L):
        mean = gs[:, 2 * l] / N
        var = gs[:, 2 * l + 1] / N - mean * mean
        a = gamma[l] / np.sqrt(var + BN_EPS)
        cvec = beta[l] - a * mean
        for c in range(NCORES):
            raw = res[c]["pools"][:, l * F:(l + 1) * F]
            g0, sp = metas[c]['g_base'], metas[c]['span']
            cnt = metas[c]['gcnt'][:sp]
            fixed = a[None, :] * raw[:sp] + np.outer(cnt, cvec)
            out[g0:g0 + sp, l * F:(l + 1) * F] += fixed.astype(np.float32)
    return out


def kernel(x, edge_index, batch, W1, b1, W2, b2, gamma, beta):
    x = np.asarray(x, np.float32)
    edge_index = np.asarray(edge_index, np.int32)
    batch = np.asarray(batch, np.int32)

    ins, metas = preprocess(x, edge_index, batch)
    nc = build_kernel()
    in_maps = make_in_maps(ins, dict(W1=W1, W2=W2, b1=b1, b2=b2,
                                     gamma=gamma, beta=beta))

    import time as _time
    last_exc = None
    for attempt in range(3):
        try:
            res = bass_utils.run_bass_kernel_spmd(
                nc, in_maps, core_ids=list(range(NCORES)))
            break
        except Exception as e:
            last_exc = e
            _time.sleep(20)
    else:
        raise last_exc

    return combine(res.results, metas, gamma, beta)


if __name__ == "__main__":
    import reference
    inputs = reference.setup_inputs()
    inputs = {k: np.asarray(v) for k, v in inputs.items()}
    got = kernel(**inputs)
    print("kernel output shape:", got.shape)


# revision 81
# speedup vs baseline: 1.1955x; 1.0049x over previous
"""GIN encoder (3x GINConv+BN + per-layer global_add_pool) on 8 TRN2 cores.

v3: sharded design. Each core owns one segment of N/8 nodes (104 tiles of
125 seats) and the edges incident to them; per layer it gathers neighbor
rows from a full replicated node table in its DRAM, computes the GIN MLP,
and contributes its shard of the next layer's table via one AllGather
(3.4MB -> 27MB bf16, ~100us). BatchNorm statistics are all-reduced
([128,2] per layer) and folded lazily into the next layer's MLP exactly as
in v2: with h = a*z + c (per-feature), the GIN input
h_i + sum_j h_j = a*(z_i + sum z_j) + c*(1+deg_i), so
  mp1 = (diag(a) W1)^T t + (W1^T c) (x) (1+deg)   [rank-1 via K=1 matmul]
and pools are fixed post-hoc: pool_bn = a*pool_raw + c (x) count_g.
Per-core pools (own segment only) are combined on the host.

The feature-major self-term table and the gather indices live entirely in
SBUF; only the node-major bf16 gather table (written by AllGather) is in
DRAM.
"""
import sys
sys.path.insert(0, '/opt/trn_rl_repo')

import numpy as np
import ml_dtypes

import concourse.bass as bass
import concourse.tile as tile
from concourse import bacc, mybir, library_config
from concourse import bass_utils

NCORES = 8
N = 100000
F = 128
E = 1600000
L = 3
NUM_GRAPHS = 512
BN_EPS = 1e-5
P = 128

SEG = 8                 # segments == cores
NPS = N // SEG          # nodes per segment (12500)
T = 104                 # dst tiles per segment
SEATS = 125             # real seats per tile
CAPB = 512              # slots per (tile, bank)
BCPT = CAPB // 128      # 128-chunks per (tile, bank)
G = 2                   # tiles per gather batch
GRP = T // G            # gather groups per core
IDXC = (G * CAPB) // 16  # gidx cols per call
COLS = T * P            # padded node columns per segment (13312)
SHR = COLS + 4          # shard rows: nodes + 4 bf16 stat rows (hi/lo f32 split)
ROWS = SEG * SHR        # full table rows
NBANK = 4
BANK_ROWS = ROWS // NBANK
assert BANK_ROWS < 32767 and T % G == 0 and T * SEATS >= NPS


def preprocess(x, edge_index, batch):
    src = edge_index[0].astype(np.int64)
    dst = edge_index[1].astype(np.int64)
    seg_of = dst // NPS
    src_bank = src // (2 * NPS)

    tile_of_g = np.empty(N, np.int64)
    seat_of_g = np.empty(N, np.int64)
    for s in range(SEG):
        m = seg_of == s
        d_loc = dst[m] - s * NPS
        counts = np.zeros((NPS, 4), np.int64)
        np.add.at(counts, (d_loc, src_bank[m]), 1)
        deg = counts.sum(1)
        order = np.argsort(-deg, kind='stable')
        rem = np.full((T, 4), CAPB, np.int64)
        seats = np.full(T, SEATS, np.int64)
        tile_of = np.full(NPS, -1, np.int64)
        seat_of = np.full(NPS, -1, np.int64)
        for d in order:
            v = counts[d]
            feas = (seats > 0) & (rem >= v).all(1)
            assert feas.any(), "tile packing failed"
            slack = (rem - v).min(1).astype(np.float64)
            slack[~feas] = -1e18
            t = int(np.argmax(slack))
            tile_of[d] = t
            seat_of[d] = SEATS - seats[t]
            rem[t] -= v
            seats[t] -= 1
        tile_of_g[s * NPS:(s + 1) * NPS] = tile_of
        seat_of_g[s * NPS:(s + 1) * NPS] = seat_of

    perm_pos = tile_of_g * P + seat_of_g                  # pos within segment
    perm_row = (np.arange(N) // NPS) * SHR + perm_pos     # global table row

    xf = np.asarray(x, np.float32)
    x_nm = np.zeros((ROWS, F), np.float32)
    x_nm[perm_row] = xf

    indeg = np.bincount(dst, minlength=N).astype(np.float32)
    d_all = np.zeros(ROWS, np.float32)
    d_all[perm_row] = 1.0 + indeg

    in_maps, metas = [], []
    for s in range(SEG):
        lo, hi = s * NPS, (s + 1) * NPS
        g0 = int(batch[lo]); sp = int(batch[hi - 1]) - g0 + 1
        assert sp <= P
        rl, rh = s * SHR, s * SHR + COLS

        brel = np.full(COLS, -1.0, np.float32)
        brel[perm_pos[lo:hi]] = (batch[lo:hi] - g0).astype(np.float32)
        brel_h = np.ascontiguousarray(brel.reshape(T, P).T)          # [128, T]

        gcnt = np.bincount(batch[lo:hi] - g0, minlength=P).astype(np.float32)
        cnt_h = gcnt.reshape(1, P)

        # edge slots for this core: key = (tile, bank)
        m = seg_of == s
        key = tile_of_g[dst[m]] * 4 + src_bank[m]
        order_e = np.argsort(key, kind='stable')
        key_s = key[order_e]
        cnt_e = np.bincount(key_s, minlength=T * 4)
        assert cnt_e.max() <= CAPB
        cstart = np.zeros(T * 4, np.int64)
        cstart[1:] = np.cumsum(cnt_e)[:-1]
        ne = int(m.sum())
        within = np.arange(ne) - np.repeat(cstart, cnt_e)
        slot = key_s * CAPB + within
        gidx_flat = np.zeros(T * 4 * CAPB, np.int64)
        drel_flat = np.full(T * 4 * CAPB, -1.0, np.float32)
        src_s = src[m][order_e]
        dst_s = dst[m][order_e]
        gidx_flat[slot] = perm_row[src_s] % BANK_ROWS
        drel_flat[slot] = seat_of_g[dst_s]

        # gidx repacked for batched calls: call = (grp, bank) over G tiles
        gi = gidx_flat.reshape(GRP, G, 4, CAPB)
        gi = gi.transpose(0, 2, 1, 3).reshape(GRP * 4, G * CAPB)
        w = gi.reshape(-1, (G * CAPB) // 16, 16).transpose(0, 2, 1)
        w = w.reshape(GRP * 4, 16, (G * CAPB) // 16)
        w = np.concatenate([w[i] for i in range(w.shape[0])], axis=1)
        gidx_h = np.tile(w, (8, 1)).astype(np.int16)     # [128, GRP*4*IDXC]
        drel_h = np.ascontiguousarray(
            drel_flat.reshape(T * 4 * BCPT, P).T)        # [128, T*4*BCPT]

        in_maps.append(dict(
            xsh=x_nm[s * SHR:(s + 1) * SHR].astype(ml_dtypes.bfloat16),
            xfm=np.ascontiguousarray(
                x_nm[rl:rh].T).astype(ml_dtypes.bfloat16),
            gidx=gidx_h, dneg=-drel_h, brel=brel_h,
            drow=d_all[rl:rh].reshape(1, COLS),
        ))
        metas.append(dict(g_base=g0, span=sp, gcnt=gcnt))
    return in_maps, metas


def build_kernel(repeat=1, skip_gather=False, skip_select=False,
                 skip_comm=False, light_select=False, fat_desc=False):
    dt = mybir.dt
    nc = bacc.Bacc("TRN2", target_bir_lowering=False, debug=False,
                   enable_asserts=False, num_devices=NCORES,
                   num_swdge_queues=4)

    xsh_d = nc.dram_tensor("xsh", [SHR, F], dt.bfloat16, kind="ExternalInput")
    xfm_d = nc.dram_tensor("xfm", [P, COLS], dt.bfloat16, kind="ExternalInput")
    gidx_d = nc.dram_tensor("gidx", [P, GRP * 4 * IDXC], dt.int16,
                            kind="ExternalInput")
    dneg_d = nc.dram_tensor("dneg", [P, T * 4 * BCPT], dt.float32,
                            kind="ExternalInput")
    ineg_d = nc.dram_tensor("ineg", [P, P], dt.float32, kind="ExternalInput")
    brel_d = nc.dram_tensor("brel", [P, T], dt.float32, kind="ExternalInput")
    drow_d = nc.dram_tensor("drow", [1, COLS], dt.float32, kind="ExternalInput")
    w1_d = nc.dram_tensor("w1", [P, L * F], dt.float32, kind="ExternalInput")
    w2_d = nc.dram_tensor("w2", [P, L * F], dt.float32, kind="ExternalInput")
    bias_d = nc.dram_tensor("bias", [P, 4 * L], dt.float32, kind="ExternalInput")
    iota_d = nc.dram_tensor("iotat", [P, P], dt.float32, kind="ExternalInput")
    ident_d = nc.dram_tensor("ident", [P, P], dt.float32, kind="ExternalInput")
    identb_d = nc.dram_tensor("identb", [P, P], dt.bfloat16,
                              kind="ExternalInput")
    selst_d = nc.dram_tensor("selst", [4 * SEG, 2], dt.bfloat16,
                             kind="ExternalInput")
    pools_d = nc.dram_tensor("pools", [P, L * F], dt.float32,
                             kind="ExternalOutput")
    stats_d = nc.dram_tensor("stats", [P, 2 * L], dt.float32,
                             kind="ExternalOutput")

    # full node tables (AllGather outputs; ping-pong across layers)
    tabs = [nc.dram_tensor(f"tab{i}", [ROWS, F], dt.bfloat16, kind="Internal",
                           addr_space="Shared") for i in range(2)]
    # own-shard bounce buffers (AllGather inputs); last 4 rows carry the
    # layer's BN partial sums as a bf16 hi/lo split of the f32 values
    tabfat = (nc.dram_tensor("tabfat", [ROWS // 2, 2 * F], dt.bfloat16,
                             kind="Internal") if fat_desc else None)
    sh_x = nc.dram_tensor("shx", [SHR, F], dt.bfloat16, kind="Internal")
    shs = [nc.dram_tensor(f"sh{l}", [SHR, F], dt.bfloat16, kind="Internal")
           for l in range(L - 1)]

    inv_n = 1.0 / N
    grp8 = [list(range(NCORES))]

    with tile.TileContext(nc) as tc:
        with tc.tile_pool(name="big", bufs=1) as big, \
             tc.tile_pool(name="gpool", bufs=3) as gpool, \
             tc.tile_pool(name="spool", bufs=24) as spool, \
             tc.tile_pool(name="work", bufs=5) as work, \
             tc.tile_pool(name="stat", bufs=1) as statp, \
             tc.tile_pool(name="psA", bufs=2, space="PSUM") as psA, \
             tc.tile_pool(name="psM", bufs=2, space="PSUM") as psM, \
             tc.tile_pool(name="psT", bufs=2, space="PSUM") as psT, \
             tc.tile_pool(name="psP", bufs=1, space="PSUM") as psP:

            nc.gpsimd.load_library(library_config.mlp)

            gidx_t = big.tile([P, GRP * 4 * IDXC], dt.int16)
            nc.sync.dma_start(gidx_t[:], gidx_d.ap())
            dneg_t = big.tile([P, T * 4 * BCPT], dt.float32)
            nc.sync.dma_start(dneg_t[:], dneg_d.ap())
            ineg_t = big.tile([P, P], dt.float32)
            nc.sync.dma_start(ineg_t[:], ineg_d.ap())
            brel_t = big.tile([P, T], dt.float32)
            nc.sync.dma_start(brel_t[:], brel_d.ap())
            drow_t = big.tile([1, COLS], dt.float32)
            nc.sync.dma_start(drow_t[:], drow_d.ap())
            selst_t = big.tile([4 * SEG, 2], dt.bfloat16)
            nc.sync.dma_start(selst_t[:], selst_d.ap())
            w1_t = big.tile([P, L * F], dt.float32)
            nc.sync.dma_start(w1_t[:], w1_d.ap())
            w2_t = big.tile([P, L * F], dt.float32)
            nc.sync.dma_start(w2_t[:], w2_d.ap())
            bias_t = big.tile([P, 4 * L], dt.float32)
            nc.sync.dma_start(bias_t[:], bias_d.ap())
            iota_t = big.tile([P, P], dt.float32)
            nc.sync.dma_start(iota_t[:], iota_d.ap())
            ident_t = big.tile([P, P], dt.float32)
            nc.sync.dma_start(ident_t[:], ident_d.ap())
            identb_t = big.tile([P, P], dt.bfloat16)
            nc.sync.dma_start(identb_t[:], identb_d.ap())

            # SBUF-resident feature-major self-term tables (ping-pong)
            hfm = [big.tile([P, COLS], dt.bfloat16, name=f"hfm{i}")
                   for i in range(2)]
            nc.sync.dma_start(hfm[0][:], xfm_d.ap())
            # stage own x shard for the layer-0 AllGather
            nc.sync.dma_start(sh_x.ap(), xsh_d.ap())

            # folded W1 and rank-1 rows for layers 1,2 (layer 0: a=1, c=0)
            w1f_t = big.tile([P, (L - 1) * F], dt.float32)
            w1c_t = big.tile([1, (L - 1) * F], dt.float32)
            ac_t = big.tile([P, 2 * L], dt.float32)
            # phantom-seat z column per layer boundary (p_{-1} = 0); phantom
            # seats flow unmasked through the MLP, their stats contribution
            # (NPH copies of an exactly-reproducible constant column) is
            # subtracted from the BN sums instead of masking every tile.
            pz_t = big.tile([P, 1], dt.float32)
            nc.vector.tensor_scalar(out=pz_t[:], in0=iota_t[:, 0:1],
                                    scalar1=0.0, scalar2=None,
                                    op0=mybir.AluOpType.mult)

            for rep in range(repeat):
              if not skip_comm:
                nc.gpsimd.collective_compute(
                    "AllGather", mybir.AluOpType.bypass, replica_groups=grp8,
                    ins=[sh_x.ap().opt()], outs=[tabs[0].ap().opt()])
              for l in range(L):
                tab = tabs[l % 2]
                hcur = hfm[l % 2]
                hnxt = hfm[(l + 1) % 2]
                b1c = bias_t[:, 0 * L + l:0 * L + l + 1]
                b2c = bias_t[:, 1 * L + l:1 * L + l + 1]
                w2c = w2_t[:, l * F:(l + 1) * F]
                w1c = (w1_t[:, 0:F] if l == 0
                       else w1f_t[:, (l - 1) * F:l * F])

                if l > 0:
                    # decode layer (l-1) global BN stats from the stat rows
                    # that rode the AllGather; fold into this layer's MLP
                    lm = l - 1
                    gac = bias_t[:, 2 * L + lm:2 * L + lm + 1]
                    bec = bias_t[:, 3 * L + lm:3 * L + lm + 1]
                    stg = work.tile([4 * SEG, P], dt.bfloat16, tag="stg",
                                    bufs=2)
                    for cc in range(SEG):
                        nc.sync.dma_start(
                            stg[4 * cc:4 * cc + 4, :],
                            tab.ap()[cc * SHR + COLS:cc * SHR + COLS + 4, :])
                    gps = psM.tile([P, P], dt.float32, tag="mp")
                    nc.tensor.matmul(gps[0:2, :], lhsT=selst_t[:],
                                     rhs=stg[:], start=True, stop=True)
                    gsb = work.tile([2, P], dt.float32, tag="gsb")
                    nc.scalar.copy(gsb[:], gps[0:2, :])
                    gtp = psT.tile([P, P], dt.float32, tag="tt")
                    nc.tensor.matmul(gtp[:, 0:2], lhsT=gsb[:],
                                     rhs=ident_t[0:2, 0:2],
                                     start=True, stop=True)
                    gred = work.tile([P, 2], dt.float32, tag="gred")
                    nc.scalar.copy(gred[:], gtp[:, 0:2])

                    mean = work.tile([P, 1], dt.float32, tag="mean")
                    nc.vector.tensor_scalar(out=mean[:], in0=gred[:, 0:1],
                                            scalar1=inv_n, scalar2=None,
                                            op0=mybir.AluOpType.mult)
                    var = work.tile([P, 1], dt.float32, tag="var")
                    nc.vector.tensor_scalar(out=var[:], in0=gred[:, 1:2],
                                            scalar1=inv_n, scalar2=None,
                                            op0=mybir.AluOpType.mult)
                    msq = work.tile([P, 1], dt.float32, tag="msq")
                    nc.vector.tensor_tensor(out=msq[:], in0=mean[:],
                                            in1=mean[:],
                                            op=mybir.AluOpType.mult)
                    nc.vector.tensor_tensor(out=var[:], in0=var[:],
                                            in1=msq[:],
                                            op=mybir.AluOpType.subtract)
                    nc.vector.tensor_scalar(out=var[:], in0=var[:],
                                            scalar1=BN_EPS, scalar2=None,
                                            op0=mybir.AluOpType.add)
                    sd = work.tile([P, 1], dt.float32, tag="sd")
                    nc.scalar.activation(sd[:], var[:],
                                         mybir.ActivationFunctionType.Sqrt)
                    inv = work.tile([P, 1], dt.float32, tag="inv")
                    nc.vector.reciprocal(inv[:], sd[:])
                    a_c = ac_t[:, 2 * lm:2 * lm + 1]
                    c_c = ac_t[:, 2 * lm + 1:2 * lm + 2]
                    nc.vector.tensor_tensor(out=a_c, in0=inv[:], in1=gac,
                                            op=mybir.AluOpType.mult)
                    tmpc = work.tile([P, 1], dt.float32, tag="tmpc")
                    nc.vector.tensor_tensor(out=tmpc[:], in0=mean[:],
                                            in1=a_c,
                                            op=mybir.AluOpType.mult)
                    nc.vector.tensor_tensor(out=c_c, in0=bec, in1=tmpc[:],
                                            op=mybir.AluOpType.subtract)
                    # W1' = diag(a) W1_l;  w1c_row = (W1_l^T c)^T
                    nc.scalar.activation(
                        w1f_t[:, lm * F:(lm + 1) * F],
                        w1_t[:, l * F:(l + 1) * F],
                        mybir.ActivationFunctionType.Identity,
                        scale=a_c)
                    w1cc = psT.tile([P, P], dt.float32, tag="tt")
                    nc.tensor.matmul(w1cc[:, 0:1],
                                     lhsT=w1_t[:, l * F:(l + 1) * F],
                                     rhs=c_c, start=True, stop=True)
                    w1cs = work.tile([P, 1], dt.float32, tag="w1cs")
                    nc.scalar.copy(w1cs[:], w1cc[:, 0:1])
                    w1cr = psT.tile([P, P], dt.float32, tag="tt")
                    nc.tensor.matmul(w1cr[0:1, :], lhsT=w1cs[:],
                                     rhs=ident_t[:], start=True, stop=True)
                    nc.scalar.copy(w1c_t[:, lm * F:(lm + 1) * F],
                                   w1cr[0:1, :])

                ssum = statp.tile([P, T], dt.float32, tag=f"ssum{l}")
                ssq = statp.tile([P, T], dt.float32, tag=f"ssq{l}")
                pool_ps = psP.tile([P, P], dt.float32, tag="pool")

                for g in range(GRP):
                    if fat_desc:
                        g_t = gpool.tile([P, 4, (G * BCPT) // 2, 2 * P],
                                         dt.bfloat16, tag="G")
                    else:
                        g_t = gpool.tile([P, 4, G * BCPT, P], dt.bfloat16,
                                         tag="G")
                    call0 = g * 4
                    if skip_gather:
                        # keep the tile allocated; trivial write
                        nc.vector.tensor_copy(g_t[:, 0, 0, 0:2],
                                              iota_t[:, 0:2])
                    if not skip_gather:
                        for b in range(4):
                            if fat_desc:
                                # TIMING DIAGNOSTIC ONLY: half the descs at
                                # 2x size (wrong data, in-bounds reads)
                                nc.gpsimd.dma_gather(
                                    out_ap=g_t[:, b],
                                    in_ap=tabfat.ap()[0:BANK_ROWS // 2, :],
                                    idxs_ap=gidx_t[:, (call0 + b) * IDXC:
                                                   (call0 + b) * IDXC
                                                   + IDXC // 2],
                                    num_idxs=(G * CAPB) // 2,
                                    num_idxs_reg=(G * CAPB) // 2,
                                    elem_size=2 * F,
                                    queue_num=b,
                                )
                                continue
                            nc.gpsimd.dma_gather(
                                out_ap=g_t[:, b],
                                in_ap=tab.ap()[b * BANK_ROWS:
                                               (b + 1) * BANK_ROWS, :],
                                idxs_ap=gidx_t[:, (call0 + b) * IDXC:
                                               (call0 + b + 1) * IDXC],
                                num_idxs=G * CAPB,
                                num_idxs_reg=G * CAPB,
                                elem_size=F,
                                queue_num=b,
                            )
                    for ti in range(G):
                        t = g * G + ti
                        aggT = psA.tile([P, P], dt.float32, tag="agg")
                        s_t = None
                        for b in range(4):
                            for sub in range(BCPT):
                                ch = t * 16 + b * BCPT + sub
                                k = ti * BCPT + sub
                                if skip_select and not (b == 0 and sub == 0):
                                    continue
                                if s_t is None or not light_select:
                                    s_t = spool.tile([P, P], dt.bfloat16,
                                                     tag="S")
                                    nc.vector.tensor_scalar(
                                        out=s_t[:], in0=ineg_t[:],
                                        scalar1=dneg_t[:, ch:ch + 1],
                                        scalar2=None,
                                        op0=mybir.AluOpType.is_equal)
                                glhs = (g_t[:, b, k // 2,
                                            (k % 2) * P:(k % 2 + 1) * P]
                                        if fat_desc else g_t[:, b, k, :])
                                nc.tensor.matmul(
                                    aggT[:], lhsT=glhs,
                                    rhs=s_t[:],
                                    start=(b == 0 and sub == 0),
                                    stop=False)
                        # += h_i via identity matmul: z1in lands in PSUM
                        nc.tensor.matmul(
                            aggT[:], lhsT=identb_t[:],
                            rhs=hcur[:, t * P:(t + 1) * P],
                            start=False, stop=True)
                        z1in = work.tile([P, P], dt.float32, tag="z1in")
                        nc.scalar.copy(z1in[:], aggT[:])
                        mp1 = psM.tile([P, P], dt.float32, tag="mp")
                        nc.tensor.matmul(mp1[:], lhsT=w1c, rhs=z1in[:],
                                         start=True, stop=(l == 0))
                        if l > 0:
                            nc.tensor.matmul(
                                mp1[:],
                                lhsT=w1c_t[:, (l - 1) * F:l * F],
                                rhs=drow_t[0:1, t * P:(t + 1) * P],
                                start=False, stop=True)
                        z1 = work.tile([P, P], dt.float32, tag="z1")
                        nc.scalar.activation(
                            z1[:], mp1[:],
                            mybir.ActivationFunctionType.Relu, bias=b1c)
                        mp2 = psM.tile([P, P], dt.float32, tag="mp")
                        nc.tensor.matmul(mp2[:], lhsT=w2c, rhs=z1[:],
                                         start=True, stop=True)
                        zf = work.tile([P, P], dt.float32, tag="zf")
                        nc.scalar.activation(
                            zf[:], mp2[:],
                            mybir.ActivationFunctionType.Relu, bias=b2c)
                        # phantom seats stay unmasked (corrected in stats).
                        # ssum rides the hfm copy's accum_out so the DVE
                        # queue holds only dependency-free selects (no
                        # head-of-line stall on zf).
                        sqs = work.tile([P, P], dt.bfloat16, tag="sqs")
                        nc.scalar.activation(
                            sqs[:], zf[:],
                            mybir.ActivationFunctionType.Square,
                            accum_out=ssq[:, t:t + 1])
                        nc.scalar.activation(
                            hnxt[:, t * P:(t + 1) * P], zf[:],
                            mybir.ActivationFunctionType.Identity,
                            accum_out=ssum[:, t:t + 1])
                        zT = psT.tile([P, P], dt.float32, tag="tt")
                        nc.tensor.transpose(zT[:], zf[:], ident_t[:])
                        znm = work.tile([P, P], dt.bfloat16, tag="znm")
                        nc.scalar.copy(znm[:], zT[:])
                        if l < L - 1:
                            nc.sync.dma_start(
                                shs[l].ap()[t * P:(t + 1) * P, :], znm[:])
                        sb_t = spool.tile([P, P], dt.bfloat16, tag="S")
                        nc.vector.tensor_scalar(
                            out=sb_t[:], in0=iota_t[:],
                            scalar1=brel_t[:, t:t + 1], scalar2=None,
                            op0=mybir.AluOpType.is_equal)
                        nc.tensor.matmul(pool_ps[:], lhsT=sb_t[:],
                                         rhs=znm[:],
                                         start=(t == 0), stop=(t == T - 1),
                                         skip_group_check=True)

                # raw pool (graph-major) for this layer
                prm = statp.tile([P, P], dt.float32, tag=f"prm{l}")
                nc.scalar.copy(prm[:], pool_ps[:])

                # phantom z column: exactly the per-tile value at a phantom
                # seat, so NPH * pz / NPH * pz^2 corrects the sums. The tile
                # path reads the previous layer's z through bf16 hfm, so
                # round-trip pz through bf16 to match bitwise.
                pzb = work.tile([P, 1], dt.bfloat16, tag="pzb")
                nc.scalar.copy(pzb[:], pz_t[:])
                pzf = work.tile([P, 1], dt.float32, tag="pzf")
                nc.scalar.copy(pzf[:], pzb[:])
                mp1p = psM.tile([P, 1], dt.float32, tag="mpp", bufs=1)
                nc.tensor.matmul(mp1p[:], lhsT=w1c, rhs=pzf[:],
                                 start=True, stop=True)
                z1p = work.tile([P, 1], dt.float32, tag="z1p")
                nc.scalar.activation(z1p[:], mp1p[:],
                                     mybir.ActivationFunctionType.Relu,
                                     bias=b1c)
                mp2p = psM.tile([P, 1], dt.float32, tag="mpp", bufs=1)
                nc.tensor.matmul(mp2p[:], lhsT=w2c, rhs=z1p[:],
                                 start=True, stop=True)
                nc.scalar.activation(pz_t[:], mp2p[:],
                                     mybir.ActivationFunctionType.Relu,
                                     bias=b2c)

                # raw (unnormalized) pool straight to the host
                nc.sync.dma_start(
                    pools_d.ap()[:, l * F:(l + 1) * F], prm[:])

                # ---- local BN partial sums; ship hi/lo split on the AG ----
                NPH = float(T * P - NPS)
                red = work.tile([P, 2], dt.float32, tag="red")
                nc.vector.tensor_reduce(out=red[:, 0:1], in_=ssum[:],
                                        axis=mybir.AxisListType.X,
                                        op=mybir.AluOpType.add)
                nc.vector.tensor_reduce(out=red[:, 1:2], in_=ssq[:],
                                        axis=mybir.AxisListType.X,
                                        op=mybir.AluOpType.add)
                pcor = work.tile([P, 2], dt.float32, tag="pcor")
                nc.vector.tensor_scalar(out=pcor[:, 0:1], in0=pz_t[:],
                                        scalar1=-NPH, scalar2=None,
                                        op0=mybir.AluOpType.mult)
                psq = work.tile([P, 1], dt.float32, tag="psq")
                nc.vector.tensor_tensor(out=psq[:], in0=pz_t[:], in1=pz_t[:],
                                        op=mybir.AluOpType.mult)
                nc.vector.tensor_scalar(out=pcor[:, 1:2], in0=psq[:],
                                        scalar1=-NPH, scalar2=None,
                                        op0=mybir.AluOpType.mult)
                nc.vector.tensor_tensor(out=red[:], in0=red[:], in1=pcor[:],
                                        op=mybir.AluOpType.add)
                nc.sync.dma_start(stats_d.ap()[:, 2 * l:2 * l + 2], red[:])

                if l < L - 1:
                    # red^T as two f32 rows -> bf16 hi/lo rows in the shard
                    redT = psT.tile([P, P], dt.float32, tag="tt")
                    nc.tensor.matmul(redT[0:2, :], lhsT=red[:],
                                     rhs=ident_t[:], start=True, stop=True)
                    redT_sb = work.tile([2, P], dt.float32, tag="redTsb")
                    nc.scalar.copy(redT_sb[:], redT[0:2, :])
                    hi_b = work.tile([2, P], dt.bfloat16, tag="hib")
                    nc.scalar.copy(hi_b[:], redT_sb[:])
                    hi_f = work.tile([2, P], dt.float32, tag="hif")
                    nc.scalar.copy(hi_f[:], hi_b[:])
                    lo_f = work.tile([2, P], dt.float32, tag="lof")
                    nc.vector.tensor_tensor(out=lo_f[:], in0=redT_sb[:],
                                            in1=hi_f[:],
                                            op=mybir.AluOpType.subtract)
                    lo_b = work.tile([2, P], dt.bfloat16, tag="lob")
                    nc.vector.tensor_copy(lo_b[:], lo_f[:])
                    nc.sync.dma_start(
                        shs[l].ap()[COLS:COLS + 2, :], hi_b[:])
                    nc.sync.dma_start(
                        shs[l].ap()[COLS + 2:COLS + 4, :], lo_b[:])
                    if not skip_comm:
                        nc.gpsimd.collective_compute(
                            "AllGather", mybir.AluOpType.bypass,
                            replica_groups=grp8,
                            ins=[shs[l].ap().opt()],
                            outs=[tabs[(l + 1) % 2].ap().opt()])

    nc.compile()
    return nc


def make_in_maps(ins, inputs):
    W1 = np.asarray(inputs['W1'], np.float32)
    W2 = np.asarray(inputs['W2'], np.float32)
    b1 = np.asarray(inputs['b1'], np.float32)
    b2 = np.asarray(inputs['b2'], np.float32)
    gamma = np.asarray(inputs['gamma'], np.float32)
    beta = np.asarray(inputs['beta'], np.float32)
    w1_h = np.ascontiguousarray(np.concatenate([W1[i] for i in range(L)], 1))
    w2_h = np.ascontiguousarray(np.concatenate([W2[i] for i in range(L)], 1))
    bias_h = np.ascontiguousarray(
        np.concatenate([b1.T, b2.T, gamma.T, beta.T], 1))
    iota_h = np.tile(np.arange(P, dtype=np.float32), (P, 1))
    ident_h = np.eye(P, dtype=np.float32)
    pm = np.arange(4 * SEG) % 4
    selst_h = np.stack([((pm == 0) | (pm == 2)), ((pm == 1) | (pm == 3))],
                       axis=1).astype(ml_dtypes.bfloat16)
    shared = {"w1": w1_h, "w2": w2_h, "bias": bias_h,
              "iotat": iota_h, "ineg": -iota_h, "ident": ident_h,
              "identb": ident_h.astype(ml_dtypes.bfloat16),
              "selst": selst_h}
    return [{**ins[c], **shared} for c in range(NCORES)]


def combine(res, metas, gamma, beta):
    """Global BN stats from per-core partial sums; fix raw pools; combine."""
    gamma = np.asarray(gamma, np.float32)
    beta = np.asarray(beta, np.float32)
    gs = np.zeros((P, 2 * L), np.float64)
    for c in range(NCORES):
        gs += res[c]["stats"].astype(np.float64)
    out = np.zeros((NUM_GRAPHS, L * F), np.float32)
    for l in range(L):
        mean = gs[:, 2 * l] / N
        var = gs[:, 2 * l + 1] / N - mean * mean
        a = gamma[l] / np.sqrt(var + BN_EPS)
        cvec = beta[l] - a * mean
        for c in range(NCORES):
            raw = res[c]["pools"][:, l * F:(l + 1) * F]
            g0, sp = metas[c]['g_base'], metas[c]['span']
            cnt = metas[c]['gcnt'][:sp]
            fixed = a[None, :] * raw[:sp] + np.outer(cnt, cvec)
            out[g0:g0 + sp, l * F:(l + 1) * F] += fixed.astype(np.float32)
    return out


def kernel(x, edge_index, batch, W1, b1, W2, b2, gamma, beta):
    x = np.asarray(x, np.float32)
    edge_index = np.asarray(edge_index, np.int32)
    batch = np.asarray(batch, np.int32)

    ins, metas = preprocess(x, edge_index, batch)
    nc = build_kernel()
    in_maps = make_in_maps(ins, dict(W1=W1, W2=W2, b1=b1, b2=b2,
                                     gamma=gamma, beta=beta))

    import time as _time
    last_exc = None
    for attempt in range(3):
        try:
            res = bass_utils.run_bass_kernel_spmd(
                nc, in_maps, core_ids=list(range(NCORES)))
            break
        except Exception as e:
            last_exc = e
            _time.sleep(20)
    else:
        raise last_exc

    return combine(res.results, metas, gamma, beta)


if __name__ == "__main__":
    import reference
    inputs = reference.setup_inputs()
    inputs = {k: np.asarray(v) for k, v in inputs.items()}
    got = kernel(**inputs)
    print("kernel output shape:", got.shape)
